# revision 31
# baseline (speedup 1.0000x reference)
"""Trainium2 Bass kernel v2: multi-head attention (B=2, T=2048, C=2048, H=16, D=128).

Sharding: tensor-parallel over heads. 8 cores x 2 heads each.
  - W_qkv columns sliced per head-pair, W_proj rows sliced per head-pair.
  - Each core computes a partial output [B*T, C]; host sums the 8 partials.

v3 changes vs v2 (435us):
  - out DMA'd straight from PSUM (f32) on the gpsimd queue: kills 256
    out-casts (~170us DVE) and moves DIRECT2D dispatch off the Scalar
    queue so exp ACTs never queue behind store dispatch.
  - finalize decoupled: unnormalized y copied out of PSUM immediately
    (ACT), reciprocal chain runs off the critical path, normalize is a
    late in-place bf16 DVE mul.  Next qt's MM2s no longer wait on the
    reciprocal DMA roundtrip.
  - dacc quads folded 4->1 on DVE before the ones-matmul: denominator
    contraction is 512 PE rows instead of 2048 (-10us PE).
  - bf16 rope tables (halves table DMA), wp DMA deferred past P1.
  - progressive ck-parts for the first proj tile so the first matmul
    only waits on the first w/x chunk DMAs, not 8 of them.
"""

import math

import numpy as np

N_CORES = 8
B, T, C = 2, 2048, 2048
N_HEAD, D = 16, 128
HPC = N_HEAD // N_CORES          # heads per core
JC = HPC * D                     # per-core slice width of qkv/proj dims

RP = 256                         # proj token tile (moving free dim)
RT = 512                         # attention query tile
KB = 128                         # key block (contraction tile)

# rope table dtype: "bf16" saves SBUF + DMA, needs mixed-dtype TT support
TABLE_DT = "bf16"


def _build(Bp, Tp, Cp, hpc, d):
    import concourse.bacc as bacc
    import concourse.tile as tile
    from concourse import mybir

    f32 = mybir.dt.float32
    bf16 = mybir.dt.bfloat16
    Exp = mybir.ActivationFunctionType.Exp
    Ln = mybir.ActivationFunctionType.Ln
    Copy = mybir.ActivationFunctionType.Copy

    jc = hpc * d
    BT = Bp * Tp
    n_ck = Cp // 128             # contraction chunks for proj
    n_rt = Tp // RP              # proj token tiles per batch
    n_sub = RP // 128            # v sub-blocks per proj tile
    n_kb = Tp // KB              # key blocks per batch
    n_kbp = n_kb // 2            # key-block pairs
    n_qt = Tp // RT              # query tiles per batch
    n_rb = Tp // 128             # row blocks for out proj
    n_ot = Cp // RT              # output column tiles
    scale = 1.0 / math.sqrt(d)
    hd = d // 2
    tdt = bf16 if TABLE_DT == "bf16" else f32

    nc = bacc.Bacc("TRN2", target_bir_lowering=False, debug=False)

    xT = nc.declare_dram_parameter("xT", [Cp, BT], bf16, isOutput=False)
    wqkv = nc.declare_dram_parameter("wqkv", [Cp, 3 * jc], bf16,
                                     isOutput=False)
    wp = nc.declare_dram_parameter("wp", [jc, Cp], bf16, isOutput=False)
    ones_d = nc.declare_dram_parameter("ones", [128, 1], bf16, isOutput=False)
    cosT = nc.declare_dram_parameter("cosT", [d, Tp], tdt, isOutput=False)
    sinT = nc.declare_dram_parameter("sinT", [d, Tp], tdt, isOutput=False)
    out = nc.declare_dram_parameter("out", [BT, Cp], bf16, isOutput=True)

    with tile.TileContext(nc) as tc:
        with (
            nc.allow_low_precision(reason="bf16 staging, f32 PSUM accum"),
            tc.tile_pool(name="wpool", bufs=1) as wpool,
            tc.tile_pool(name="acts", bufs=1) as acts,
            tc.tile_pool(name="xpool", bufs=17) as xpool,
            tc.tile_pool(name="rope", bufs=4) as ropep,
            tc.tile_pool(name="epool", bufs=4) as epool,
            tc.tile_pool(name="dpool", bufs=3) as dpool,
            tc.tile_pool(name="small", bufs=2) as small,
            tc.tile_pool(name="opool", bufs=6) as opool,
            tc.tile_pool(name="pss", bufs=2, space="PSUM") as pss,
            tc.tile_pool(name="psy", bufs=1, space="PSUM") as psy,
        ):
            psp_cm = tc.tile_pool(name="psp", bufs=1, space="PSUM")
            psp = psp_cm.__enter__()
            psoA_cm = tc.tile_pool(name="psoA", bufs=1, space="PSUM")
            pso = psoA_cm.__enter__()
            # ---- resident weights / tables ----
            # inputs (w, x) stream on the sync/SP queue; tables + wp + all
            # output traffic go on the scalar queue so they never starve
            # the x-tile stream.
            wq_sb, wk_sb, wv_sb = [], [], []
            xpair = {}
            for ck in range(n_ck):
                # x before w per ck so the first matmul's sem wait covers
                # the fewest preceding queue entries
                xp = xpool.tile([128, 2 * RP], bf16, tag="xt",
                                name=f"xtpre{ck}")
                nc.sync.dma_start(xp, xT[ck * 128:(ck + 1) * 128, 0:2 * RP])
                xpair[(0, 0, ck)] = xp
                t = wpool.tile([128, 3 * jc], bf16, tag=f"w{ck}",
                               name=f"w{ck}")
                nc.sync.dma_start(t, wqkv[ck * 128:(ck + 1) * 128, :])
                wq_sb.append(t[:, 0:jc])
                wk_sb.append(t[:, jc:2 * jc])
                wv_sb.append(t[:, 2 * jc:3 * jc])
            ones_sb = wpool.tile([128, 1], bf16, tag="ones")
            nc.sync.dma_start(ones_sb, ones_d[:])
            # doubled rope tables for h-batched rope: [d, hpc, Tp]
            cos2 = wpool.tile([d, hpc, Tp], tdt, tag="cos2")
            sin2 = wpool.tile([d, hpc, Tp], tdt, tag="sin2")
            for h in range(hpc):
                nc.scalar.dma_start(cos2[:, h, :], cosT[:])
                nc.scalar.dma_start(sin2[:, h, :], sinT[:])
            wp_sb = wpool.tile([128, hpc, Cp], bf16, tag="wp")

            # per-batch activation tensors (both batches resident -> the
            # scheduler can overlap attn(b) with proj(b+1))
            qT_sb = [acts.tile([128, hpc, Tp], bf16, tag=f"qT{b}",
                               name=f"qT{b}") for b in range(Bp)]
            kT_sb = [acts.tile([128, hpc, Tp], bf16, tag=f"kT{b}",
                               name=f"kT{b}") for b in range(Bp)]
            v_sb = [acts.tile([128, n_kb, jc], bf16, tag=f"v{b}",
                              name=f"v{b}") for b in range(Bp)]
            yT_sb = [acts.tile([128, hpc, Tp], bf16, tag=f"yT{b}",
                               name=f"yT{b}") for b in range(Bp)]

            def proj_tile_range(b, rt, ck_lo, ck_hi, state):
                """qkv projection for one RP-wide token tile + rope,
                emitted in ck-range parts so attention work can weave
                between them at fine grain."""
                tsl = slice(rt * RP, (rt + 1) * RP)
                if ck_lo == 0:
                    state["q"] = psp.tile([128, hpc, RP], f32, tag="qps",
                                          name=f"qps{b}_{rt}")
                    state["k"] = psp.tile([128, hpc, RP], f32, tag="kps",
                                          name=f"kps{b}_{rt}")
                    state["v"] = psp.tile([128, n_sub, jc], f32, tag="vps",
                                          name=f"vps{b}_{rt}")
                q_ps, k_ps, v_ps = state["q"], state["k"], state["v"]
                for ck in range(ck_lo, ck_hi):
                    # x streamed in [128, 2*RP] pair-slabs (1KB descriptors)
                    pk = (b, rt // 2, ck)
                    if pk not in xpair:
                        xp = xpool.tile([128, 2 * RP], bf16, tag="xt")
                        p0 = b * Tp + (rt // 2) * 2 * RP
                        nc.sync.dma_start(
                            xp, xT[ck * 128:(ck + 1) * 128, p0:p0 + 2 * RP])
                        xpair[pk] = xp
                    xt = xpair[pk][:, (rt % 2) * RP:(rt % 2) * RP + RP]
                    first = ck == 0
                    last = ck == n_ck - 1
                    for h in range(hpc):
                        nc.tensor.matmul(
                            q_ps[:, h, :],
                            wq_sb[ck][:, h * d:(h + 1) * d],
                            xt, start=(first and h == 0),
                            stop=(last and h == hpc - 1),
                            skip_group_check=True)
                        nc.tensor.matmul(
                            k_ps[:, h, :],
                            wk_sb[ck][:, h * d:(h + 1) * d],
                            xt, start=(first and h == 0),
                            stop=(last and h == hpc - 1),
                            skip_group_check=True)
                    for s in range(n_sub):
                        nc.tensor.matmul(
                            v_ps[:, s, :],
                            xt[:, s * 128:(s + 1) * 128],
                            wv_sb[ck], start=(first and s == 0),
                            stop=(last and s == n_sub - 1),
                            skip_group_check=True)
                if ck_hi != n_ck:
                    return
                # h-batched rope epilogue on [128, hpc*RP]:
                #   dst = psum*cos2 + swap(psum)*sin2_signed
                for ps, dst in ((q_ps, qT_sb[b]), (k_ps, kT_sb[b])):
                    c2 = cos2[:, :, tsl]
                    s2 = sin2[:, :, tsl]
                    t1 = ropep.tile([d, hpc, RP], f32, tag="t1")
                    nc.vector.tensor_mul(t1, ps, c2)
                    t2 = ropep.tile([d, hpc, RP], f32, tag="t2")
                    nc.vector.tensor_mul(t2[0:hd], ps[hd:d], s2[0:hd])
                    nc.vector.tensor_mul(t2[hd:d], ps[0:hd], s2[hd:d])
                    nc.vector.tensor_add(dst[:, :, tsl], t1, t2)
                # v copy (both sub-blocks in one ACT instruction)
                nc.scalar.activation(
                    v_sb[b][:, rt * n_sub:(rt + 1) * n_sub, :], v_ps, Copy)

            def proj_tile(b, rt):
                st = {}
                for part in range(4):
                    proj_tile_range(b, rt, part * 4, (part + 1) * 4, st)

            def attn_unit(b, qt, kb, h, y_ps, daccs, equads, pend):
                """one (query-tile, key-block, head) attention step.

                s tiles are single-bank [128, RT], double-buffered -> the
                next MM1 overlaps the current exp.  e tiles are quads
                [128, 4, RT] shared by 4 consecutive key blocks so dacc
                accumulates 2048 elems per DVE op.
                """
                qsl = slice(qt * RT, (qt + 1) * RT)
                s_ps = pss.tile([128, RT], f32, tag="s")
                nc.tensor.matmul(
                    s_ps,
                    kT_sb[b][:, h, kb * KB:(kb + 1) * KB],
                    qT_sb[b][:, h, qsl],
                    start=True, stop=True, skip_group_check=True)
                if kb % 4 == 0:
                    equads[h] = epool.tile([128, 4, RT], bf16, tag="e",
                                           name=f"e{b}_{qt}_{h}_{kb}")
                eq = equads[h]
                nc.scalar.activation(eq[:, kb % 4, :], s_ps, Exp,
                                     scale=scale)
                if kb % 4 == 3:
                    # denominator accumulation on gpsimd: all-SBUF work,
                    # keeps DVE free for the PSUM-coupled casts
                    qd = kb // 4
                    if qd == 0:
                        nc.gpsimd.tensor_copy(out=daccs[h], in_=eq)
                    else:
                        nc.gpsimd.tensor_add(daccs[h], daccs[h], eq)
                pend[h].append((kb, eq))

            def mm2_drain(b, h, y_ps, pend, keep):
                while len(pend[h]) > keep:
                    kb, eq = pend[h].pop(0)
                    nc.tensor.matmul(
                        y_ps[h],
                        v_sb[b][:, kb, h * d:(h + 1) * d],
                        eq[:, kb % 4, :],
                        start=(kb == 0), stop=(kb == n_kb - 1),
                        skip_group_check=True)

            def finalize(b, qt, h, y_ps, daccs, fused=False):
                qsl = slice(qt * RT, (qt + 1) * RT)
                # free the PSUM bank immediately: unnormalized copy; the
                # reciprocal chain below runs off the critical path and
                # the normalize is a late in-place bf16 mul.  (fused path
                # for the very last tile: keep y in PSUM, one less link.)
                if not fused:
                    nc.scalar.activation(yT_sb[b][:, h, qsl], y_ps[h], Copy)
                # fold dacc quads 4->1 so the ones-matmul is 512 rows
                f2 = small.tile([128, 2, RT], bf16, tag="f2")
                nc.gpsimd.tensor_add(f2, daccs[h][:, 0:2, :],
                                     daccs[h][:, 2:4, :])
                fold = small.tile([128, RT], bf16, tag="fold")
                nc.gpsimd.tensor_add(fold, f2[:, 0, :], f2[:, 1, :])
                dsum = pso.tile([1, RT], f32, tag="fin", bufs=1,
                                name=f"dsum{b}_{qt}_{h}")
                nc.tensor.matmul(dsum, ones_sb, fold, start=True,
                                 stop=True, skip_group_check=True)
                # single-op DVE Newton reciprocal straight off PSUM (no
                # DMA-spread roundtrip); 18 correct bits, plenty for bf16
                rec = small.tile([1, RT], f32, tag="rec")
                nc.vector.reciprocal_approx_fast(out=rec, in_=dsum)
                rec_bf = small.tile([1, RT], bf16, tag="recb")
                nc.scalar.activation(rec_bf, rec, Copy)
                bc = small.tile([128, RT], bf16, tag="bc")
                nc.gpsimd.partition_broadcast(out_ap=bc, in_ap=rec_bf)
                if fused:
                    # reads PSUM -> must be DVE
                    nc.vector.tensor_mul(yT_sb[b][:, h, qsl], y_ps[h], bc)
                else:
                    nc.gpsimd.tensor_mul(yT_sb[b][:, h, qsl],
                                         yT_sb[b][:, h, qsl], bc)

            def attn_qt(b, qt, interleave=None, defer_last_fin=False,
                        fused_tail=False):
                """all attention work for one query tile; interleave is a
                list of thunks emitted between key-block units.

                h-outer: head 0 finishes halfway through the tile so its
                finalize chain (reciprocal etc.) hides under head 1's
                attention units instead of stalling the next tile.  The
                last head's finalize can be deferred (returned as a thunk
                for the caller to weave into the NEXT tile) so its dsum
                matmul never head-of-line blocks the in-order PE queue
                while the DVE fold catches up."""
                y_ps = [psy.tile([d, RT], f32, tag=f"y{h}",
                                 name=f"y{b}_{qt}_{h}")
                        for h in range(hpc)]
                daccs = [dpool.tile([128, 4, RT], bf16, tag="dacc",
                                    name=f"dacc{b}_{qt}_{h}")
                         for h in range(hpc)]
                equads = [None] * hpc
                pend = [[] for _ in range(hpc)]
                il = list(interleave or [])
                # spread the filler thunks across the unit sequence
                nu = hpc * n_kb
                points = {}
                for i, th in enumerate(il):
                    points.setdefault(min(nu - 1, i * nu // len(il)),
                                      []).append(th)
                u = 0
                for h in range(hpc):
                    for kb in range(n_kb):
                        attn_unit(b, qt, kb, h, y_ps, daccs, equads, pend)
                        # lag the MM2s two key-blocks behind their exp so
                        # the PE never head-of-line blocks on ScalarE
                        mm2_drain(b, h, y_ps, pend, 2)
                        for th in points.get(u, []):
                            th()
                        u += 1
                    mm2_drain(b, h, y_ps, pend, 0)
                    if h < hpc - 1:
                        finalize(b, qt, h, y_ps, daccs)
                hl = hpc - 1
                fin = (lambda: finalize(b, qt, hl, y_ps, daccs,
                                        fused=fused_tail))
                if defer_last_fin:
                    return fin
                fin()
                return None

            def outproj_unit(b, rb, ot, eng):
                o_ps = pso.tile([128, RT], f32, tag="o")
                for h in range(hpc):
                    nc.tensor.matmul(
                        o_ps,
                        yT_sb[b][:, h, rb * 128:(rb + 1) * 128],
                        wp_sb[:, h, ot * RT:(ot + 1) * RT],
                        start=(h == 0), stop=(h == hpc - 1),
                        skip_group_check=True)
                # cast on DVE/ACT (gpsimd has no PSUM port); the DIRECT2D
                # dispatch rides the sync queue -- idle in P3 (the x
                # stream is done) -- so exp ACTs and broadcasts never
                # queue behind store dispatch
                o_sb = opool.tile([128, RT], bf16, tag="o")
                if eng == 0:
                    nc.vector.tensor_copy(out=o_sb, in_=o_ps)
                else:
                    nc.scalar.activation(o_sb, o_ps, Copy)
                nc.sync.dma_start(
                    out[b * Tp + rb * 128:b * Tp + (rb + 1) * 128,
                        ot * RT:(ot + 1) * RT],
                    o_sb)

            # ================= emission schedule =================
            ppb = n_rt // n_qt          # proj tiles per query tile
            opb = n_rb // n_qt          # row blocks per query tile
            cnt = [0]
            # P1: proj b0 with attn(b0, qt0) streaming kb-wise behind
            # the tiles that produce its k/v blocks (fills the otherwise
            # idle ScalarE and covers DMA stalls with PE work)
            yq0 = [psy.tile([d, RT], f32, tag=f"y{h}", name=f"yq0_{h}")
                   for h in range(hpc)]
            dq0 = [dpool.tile([128, 4, RT], bf16, tag="dacc",
                              name=f"daccq0_{h}") for h in range(hpc)]
            eq0 = [None] * hpc
            pq0 = [[] for _ in range(hpc)]
            for rt in range(n_rt):
                if rt == 0:
                    # progressive ck-parts: the first matmuls wait only on
                    # the first one or two w/x chunk DMAs
                    st0 = {}
                    for lo, hi in ((0, 1), (1, 2), (2, 4), (4, 8), (8, 16)):
                        proj_tile_range(0, 0, lo, hi, st0)
                else:
                    proj_tile(0, rt)
                if rt == 1:
                    # wp is first used in P3; deferring its 1MB DMA keeps
                    # early HBM bandwidth for the w/x critical path
                    nc.scalar.dma_start(
                        wp_sb, wp.rearrange("(h p) o -> p h o", p=128))
                if rt >= 2:
                    for kb in (2 * (rt - 2), 2 * (rt - 2) + 1):
                        for h in range(hpc):
                            attn_unit(0, 0, kb, h, yq0, dq0, eq0, pq0)
                            mm2_drain(0, h, yq0, pq0, 2)
            # finish qt0's remaining key blocks, then finalize it
            for kb in range(2 * (n_rt - 2), n_kb):
                for h in range(hpc):
                    attn_unit(0, 0, kb, h, yq0, dq0, eq0, pq0)
                    mm2_drain(0, h, yq0, pq0, 2)
            for h in range(hpc):
                mm2_drain(0, h, yq0, pq0, 0)
            for h in range(hpc - 1):
                finalize(0, 0, h, yq0, dq0)
            pend_fin = lambda: finalize(0, 0, hpc - 1, yq0, dq0)
            # P2: attn b0 qt1-3 interleaved with ALL 8 proj-b1 tiles
            # (quarter-tile weave, distributed across the 3 query tiles);
            # the previous tile's deferred finalize leads each weave
            for qt in range(1, n_qt):
                j = qt - 1
                thunks = [pend_fin]
                for rt in range(j * n_rt // 3, (j + 1) * n_rt // 3):
                    st = {}
                    for part in range(4):
                        thunks.append(
                            lambda rt=rt, part=part, st=st:
                                proj_tile_range(1, rt, part * 4,
                                                (part + 1) * 4, st))
                pend_fin = attn_qt(0, qt, interleave=thunks,
                                   defer_last_fin=True)
            # proj + b0 dsums done -> release 4 banks for the P3 pool
            psoA_cm.__exit__(None, None, None)
            psp_cm.__exit__(None, None, None)
            psoB_cm = tc.tile_pool(name="psoB", bufs=3, space="PSUM")
            pso = psoB_cm.__enter__()
            # P3: attn b1; outproj b0 and b1 woven in 2-unit thunks so
            # ScalarE never starves between query tiles
            def op_thunks(units):
                ths = []
                for i in range(0, len(units), 2):
                    chunk = units[i:i + 2]
                    def th(chunk=chunk):
                        for b_, rb, ot in chunk:
                            cnt[0] += 1
                            outproj_unit(b_, rb, ot, 0)
                    ths.append(th)
                return ths
            for qt in range(n_qt):
                # b0-row stores early; b1-rows of the previous tile late
                # (they read yT written by the deferred finalize leading
                # this weave, so give its chain time to drain)
                units0 = [(0, rb, ot)
                          for rb in range(qt * opb, (qt + 1) * opb)
                          for ot in range(n_ot)]
                units1 = [(1, rb, ot)
                          for rb in range((qt - 1) * opb, qt * opb)
                          for ot in range(n_ot)] if qt > 0 else []
                thunks = [pend_fin] + op_thunks(units0) + op_thunks(units1)
                last = qt == n_qt - 1
                pend_fin = attn_qt(1, qt, interleave=thunks,
                                   defer_last_fin=not last,
                                   fused_tail=last)
            # P4: last query tile's outproj b1; alternate cast engines
            # (ACT is idle here) so the drain is not CAST-bound
            for rb in range((n_qt - 1) * opb, n_qt * opb):
                for ot in range(n_ot):
                    cnt[0] += 1
                    outproj_unit(1, rb, ot, ot % 2)
            psoB_cm.__exit__(None, None, None)

    nc.compile()
    return nc


def _prep_in_maps(x, cos, sin, W_qkv, W_proj, n_cores, hpc, d):
    """Host-side shard prep: pure layout work (transpose / slice / sign fold)."""
    Bp, Tp, Cp = x.shape
    jc = hpc * d
    import ml_dtypes
    tdt = ml_dtypes.bfloat16 if TABLE_DT == "bf16" else np.float32
    xTa = np.ascontiguousarray(x.reshape(Bp * Tp, Cp).T).astype(ml_dtypes.bfloat16)
    cosT = np.ascontiguousarray(cos.T).astype(tdt)
    sinT = np.ascontiguousarray(sin.T).copy()
    sinT[: d // 2] *= -1.0
    sinT = sinT.astype(tdt)
    in_maps = []
    for c in range(n_cores):
        j0, j1 = c * jc, (c + 1) * jc
        in_maps.append({
            "xT": xTa,
            "wqkv": np.ascontiguousarray(np.concatenate(
                [W_qkv[:, j0:j1], W_qkv[:, Cp + j0:Cp + j1],
                 W_qkv[:, 2 * Cp + j0:2 * Cp + j1]], axis=1,
            )).astype(ml_dtypes.bfloat16),
            "wp": np.ascontiguousarray(W_proj[j0:j1, :]).astype(ml_dtypes.bfloat16),
            "ones": np.ones((128, 1), dtype=ml_dtypes.bfloat16),
            "cosT": cosT,
            "sinT": sinT,
        })
    return in_maps


def _install_ntff_hook():
    """Enable NTFF profiling under axon when the boot image lacks the
    antenv.axon_hooks shim. Harmless if anything is missing."""
    import sys
    import types
    try:
        from antenv.axon_hooks import get_axon_ntff_profile_hook
        if get_axon_ntff_profile_hook() is not None:
            return
    except ImportError:
        pass
    try:
        sys.path.insert(0, "/root/.axon_site")
        from trn_agent_boot.trn_boot import _ntff_profile_via_ctypes

        hook = _ntff_profile_via_ctypes("/opt/axon/libaxon_pjrt.so")
        if hook is None:
            return
        mod = types.ModuleType("antenv.axon_hooks")
        mod.get_axon_ntff_profile_hook = lambda: hook
        mod.set_axon_ntff_profile_hook = lambda h: None
        import antenv
        antenv.axon_hooks = mod
        sys.modules["antenv.axon_hooks"] = mod
    except Exception:
        pass


def _run(x, cos, sin, W_qkv, W_proj, trace=False):
    from concourse.bass_utils import run_bass_kernel_spmd

    if trace:
        _install_ntff_hook()

    x = np.ascontiguousarray(x, dtype=np.float32)
    cos = np.ascontiguousarray(cos, dtype=np.float32)
    sin = np.ascontiguousarray(sin, dtype=np.float32)
    W_qkv = np.ascontiguousarray(W_qkv, dtype=np.float32)
    W_proj = np.ascontiguousarray(W_proj, dtype=np.float32)

    Bp, Tp, Cp = x.shape
    nc = _build(Bp, Tp, Cp, HPC, D)
    in_maps = _prep_in_maps(x, cos, sin, W_qkv, W_proj, N_CORES, HPC, D)
    res = run_bass_kernel_spmd(nc, in_maps, core_ids=list(range(N_CORES)),
                               trace=trace)
    acc = np.zeros((Bp * Tp, Cp), dtype=np.float32)
    for i in range(N_CORES):
        acc += np.asarray(res.results[i]["out"], dtype=np.float32)
    return acc.reshape(Bp, Tp, Cp), res


def kernel(x, cos, sin, W_qkv, W_proj):
    out, _ = _run(x, cos, sin, W_qkv, W_proj, trace=False)
    return out



# revision 42
# speedup vs baseline: 1.7957x; 1.7957x over previous
"""Trainium2 Bass kernel v2: multi-head attention (B=2, T=2048, C=2048, H=16, D=128).

Sharding: tensor-parallel over heads. 8 cores x 2 heads each.
  - W_qkv columns sliced per head-pair, W_proj rows sliced per head-pair.
  - Each core computes a partial output [B*T, C]; host sums the 8 partials.

v3 changes vs v2 (435us):
  - out DMA'd straight from PSUM (f32) on the gpsimd queue: kills 256
    out-casts (~170us DVE) and moves DIRECT2D dispatch off the Scalar
    queue so exp ACTs never queue behind store dispatch.
  - finalize decoupled: unnormalized y copied out of PSUM immediately
    (ACT), reciprocal chain runs off the critical path, normalize is a
    late in-place bf16 DVE mul.  Next qt's MM2s no longer wait on the
    reciprocal DMA roundtrip.
  - dacc quads folded 4->1 on DVE before the ones-matmul: denominator
    contraction is 512 PE rows instead of 2048 (-10us PE).
  - bf16 rope tables (halves table DMA), wp DMA deferred past P1.
  - progressive ck-parts for the first proj tile so the first matmul
    only waits on the first w/x chunk DMAs, not 8 of them.
"""

import math

import numpy as np

N_CORES = 8
B, T, C = 2, 2048, 2048
N_HEAD, D = 16, 128
HPC = N_HEAD // N_CORES          # heads per core
JC = HPC * D                     # per-core slice width of qkv/proj dims

RP = 256                         # proj token tile (moving free dim)
RT = 512                         # attention query tile
KB = 128                         # key block (contraction tile)

# rope table dtype: "bf16" saves SBUF + DMA, needs mixed-dtype TT support
TABLE_DT = "bf16"


def _build(Bp, Tp, Cp, hpc, d):
    import concourse.bacc as bacc
    import concourse.tile as tile
    from concourse import mybir

    f32 = mybir.dt.float32
    bf16 = mybir.dt.bfloat16
    Exp = mybir.ActivationFunctionType.Exp
    Ln = mybir.ActivationFunctionType.Ln
    Copy = mybir.ActivationFunctionType.Copy

    jc = hpc * d
    BT = Bp * Tp
    n_ck = Cp // 128             # contraction chunks for proj
    n_rt = Tp // RP              # proj token tiles per batch
    n_sub = RP // 128            # v sub-blocks per proj tile
    n_kb = Tp // KB              # key blocks per batch
    n_kbp = n_kb // 2            # key-block pairs
    n_qt = Tp // RT              # query tiles per batch
    n_rb = Tp // 128             # row blocks for out proj
    n_ot = Cp // RT              # output column tiles
    scale = 1.0 / math.sqrt(d)
    hd = d // 2
    tdt = bf16 if TABLE_DT == "bf16" else f32

    nc = bacc.Bacc("TRN2", target_bir_lowering=False, debug=False)

    xT = nc.declare_dram_parameter("xT", [Cp, BT], bf16, isOutput=False)
    wqkv = nc.declare_dram_parameter("wqkv", [Cp, 3 * jc], bf16,
                                     isOutput=False)
    wp = nc.declare_dram_parameter("wp", [jc, Cp], bf16, isOutput=False)
    ones_d = nc.declare_dram_parameter("ones", [128, 1], bf16, isOutput=False)
    cosT = nc.declare_dram_parameter("cosT", [d, Tp], tdt, isOutput=False)
    sinT = nc.declare_dram_parameter("sinT", [d, Tp], tdt, isOutput=False)
    out = nc.declare_dram_parameter("out", [BT, Cp], bf16, isOutput=True)

    with tile.TileContext(nc) as tc:
        with (
            nc.allow_low_precision(reason="bf16 staging, f32 PSUM accum"),
            tc.tile_pool(name="wpool", bufs=1) as wpool,
            tc.tile_pool(name="acts", bufs=1) as acts,
            tc.tile_pool(name="xpool", bufs=17) as xpool,
            tc.tile_pool(name="rope", bufs=4) as ropep,
            tc.tile_pool(name="epool", bufs=4) as epool,
            tc.tile_pool(name="dpool", bufs=3) as dpool,
            tc.tile_pool(name="small", bufs=2) as small,
            tc.tile_pool(name="opool", bufs=6) as opool,
            tc.tile_pool(name="pss", bufs=2, space="PSUM") as pss,
            tc.tile_pool(name="psy", bufs=1, space="PSUM") as psy,
        ):
            psp_cm = tc.tile_pool(name="psp", bufs=1, space="PSUM")
            psp = psp_cm.__enter__()
            psoA_cm = tc.tile_pool(name="psoA", bufs=1, space="PSUM")
            pso = psoA_cm.__enter__()
            # ---- resident weights / tables ----
            # inputs (w, x) stream on the sync/SP queue; tables + wp + all
            # output traffic go on the scalar queue so they never starve
            # the x-tile stream.
            wq_sb, wk_sb, wv_sb = [], [], []
            xpair = {}
            for ck in range(n_ck):
                # x before w per ck so the first matmul's sem wait covers
                # the fewest preceding queue entries
                xp = xpool.tile([128, 2 * RP], bf16, tag="xt",
                                name=f"xtpre{ck}")
                nc.sync.dma_start(xp, xT[ck * 128:(ck + 1) * 128, 0:2 * RP])
                xpair[(0, 0, ck)] = xp
                t = wpool.tile([128, 3 * jc], bf16, tag=f"w{ck}",
                               name=f"w{ck}")
                nc.sync.dma_start(t, wqkv[ck * 128:(ck + 1) * 128, :])
                wq_sb.append(t[:, 0:jc])
                wk_sb.append(t[:, jc:2 * jc])
                wv_sb.append(t[:, 2 * jc:3 * jc])
            ones_sb = wpool.tile([128, 1], bf16, tag="ones")
            nc.sync.dma_start(ones_sb, ones_d[:])
            # doubled rope tables for h-batched rope: [d, hpc, Tp]
            cos2 = wpool.tile([d, hpc, Tp], tdt, tag="cos2")
            sin2 = wpool.tile([d, hpc, Tp], tdt, tag="sin2")
            for h in range(hpc):
                nc.scalar.dma_start(cos2[:, h, :], cosT[:])
                nc.scalar.dma_start(sin2[:, h, :], sinT[:])
            wp_sb = wpool.tile([128, hpc, Cp], bf16, tag="wp")

            # per-batch activation tensors (both batches resident -> the
            # scheduler can overlap attn(b) with proj(b+1))
            qT_sb = [acts.tile([128, hpc, Tp], bf16, tag=f"qT{b}",
                               name=f"qT{b}") for b in range(Bp)]
            kT_sb = [acts.tile([128, hpc, Tp], bf16, tag=f"kT{b}",
                               name=f"kT{b}") for b in range(Bp)]
            v_sb = [acts.tile([128, n_kb, jc], bf16, tag=f"v{b}",
                              name=f"v{b}") for b in range(Bp)]
            yT_sb = [acts.tile([128, hpc, Tp], bf16, tag=f"yT{b}",
                               name=f"yT{b}") for b in range(Bp)]

            def proj_tile_range(b, rt, ck_lo, ck_hi, state):
                """qkv projection for one RP-wide token tile + rope,
                emitted in ck-range parts so attention work can weave
                between them at fine grain."""
                tsl = slice(rt * RP, (rt + 1) * RP)
                if ck_lo == 0:
                    state["q"] = psp.tile([128, hpc, RP], f32, tag="qps",
                                          name=f"qps{b}_{rt}")
                    state["k"] = psp.tile([128, hpc, RP], f32, tag="kps",
                                          name=f"kps{b}_{rt}")
                    state["v"] = psp.tile([128, n_sub, jc], f32, tag="vps",
                                          name=f"vps{b}_{rt}")
                q_ps, k_ps, v_ps = state["q"], state["k"], state["v"]
                for ck in range(ck_lo, ck_hi):
                    # x streamed in [128, 2*RP] pair-slabs (1KB descriptors)
                    pk = (b, rt // 2, ck)
                    if pk not in xpair:
                        xp = xpool.tile([128, 2 * RP], bf16, tag="xt")
                        p0 = b * Tp + (rt // 2) * 2 * RP
                        nc.sync.dma_start(
                            xp, xT[ck * 128:(ck + 1) * 128, p0:p0 + 2 * RP])
                        xpair[pk] = xp
                    xt = xpair[pk][:, (rt % 2) * RP:(rt % 2) * RP + RP]
                    first = ck == 0
                    last = ck == n_ck - 1
                    for h in range(hpc):
                        nc.tensor.matmul(
                            q_ps[:, h, :],
                            wq_sb[ck][:, h * d:(h + 1) * d],
                            xt, start=(first and h == 0),
                            stop=(last and h == hpc - 1),
                            skip_group_check=True)
                        nc.tensor.matmul(
                            k_ps[:, h, :],
                            wk_sb[ck][:, h * d:(h + 1) * d],
                            xt, start=(first and h == 0),
                            stop=(last and h == hpc - 1),
                            skip_group_check=True)
                    for s in range(n_sub):
                        nc.tensor.matmul(
                            v_ps[:, s, :],
                            xt[:, s * 128:(s + 1) * 128],
                            wv_sb[ck], start=(first and s == 0),
                            stop=(last and s == n_sub - 1),
                            skip_group_check=True)
                if ck_hi != n_ck:
                    return
                # h-batched rope epilogue on [128, hpc*RP]:
                #   dst = psum*cos2 + swap(psum)*sin2_signed
                for ps, dst in ((q_ps, qT_sb[b]), (k_ps, kT_sb[b])):
                    c2 = cos2[:, :, tsl]
                    s2 = sin2[:, :, tsl]
                    t1 = ropep.tile([d, hpc, RP], f32, tag="t1")
                    nc.vector.tensor_mul(t1, ps, c2)
                    t2 = ropep.tile([d, hpc, RP], f32, tag="t2")
                    nc.vector.tensor_mul(t2[0:hd], ps[hd:d], s2[0:hd])
                    nc.vector.tensor_mul(t2[hd:d], ps[0:hd], s2[hd:d])
                    nc.vector.tensor_add(dst[:, :, tsl], t1, t2)
                # v copy (both sub-blocks in one ACT instruction)
                nc.scalar.activation(
                    v_sb[b][:, rt * n_sub:(rt + 1) * n_sub, :], v_ps, Copy)

            def proj_tile(b, rt):
                st = {}
                for part in range(4):
                    proj_tile_range(b, rt, part * 4, (part + 1) * 4, st)

            def attn_unit(b, qt, kb, h, y_ps, daccs, equads, pend):
                """one (query-tile, key-block, head) attention step.

                s tiles are single-bank [128, RT], double-buffered -> the
                next MM1 overlaps the current exp.  e tiles are quads
                [128, 4, RT] shared by 4 consecutive key blocks so dacc
                accumulates 2048 elems per DVE op.
                """
                qsl = slice(qt * RT, (qt + 1) * RT)
                s_ps = pss.tile([128, RT], f32, tag="s")
                nc.tensor.matmul(
                    s_ps,
                    kT_sb[b][:, h, kb * KB:(kb + 1) * KB],
                    qT_sb[b][:, h, qsl],
                    start=True, stop=True, skip_group_check=True)
                if kb % 4 == 0:
                    equads[h] = epool.tile([128, 4, RT], bf16, tag="e",
                                           name=f"e{b}_{qt}_{h}_{kb}")
                eq = equads[h]
                nc.scalar.activation(eq[:, kb % 4, :], s_ps, Exp,
                                     scale=scale)
                if kb % 4 == 3:
                    qd = kb // 4
                    if qd == 0:
                        nc.vector.tensor_copy(out=daccs[h], in_=eq)
                    else:
                        nc.vector.tensor_add(daccs[h], daccs[h], eq)
                pend[h].append((kb, eq))

            def mm2_drain(b, h, y_ps, pend, keep):
                while len(pend[h]) > keep:
                    kb, eq = pend[h].pop(0)
                    nc.tensor.matmul(
                        y_ps[h],
                        v_sb[b][:, kb, h * d:(h + 1) * d],
                        eq[:, kb % 4, :],
                        start=(kb == 0), stop=(kb == n_kb - 1),
                        skip_group_check=True)

            def finalize(b, qt, h, y_ps, daccs, fused=False):
                qsl = slice(qt * RT, (qt + 1) * RT)
                # free the PSUM bank immediately: unnormalized copy; the
                # reciprocal chain below runs off the critical path and
                # the normalize is a late in-place bf16 mul.  (fused path
                # for the very last tile: keep y in PSUM, one less link.)
                if not fused:
                    nc.scalar.activation(yT_sb[b][:, h, qsl], y_ps[h], Copy)
                # fold dacc quads 4->1 so the ones-matmul is 512 rows
                f2 = small.tile([128, 2, RT], bf16, tag="f2")
                nc.vector.tensor_add(f2, daccs[h][:, 0:2, :],
                                     daccs[h][:, 2:4, :])
                fold = small.tile([128, RT], bf16, tag="fold")
                nc.vector.tensor_add(fold, f2[:, 0, :], f2[:, 1, :])
                dsum = pso.tile([1, RT], f32, tag="fin", bufs=1,
                                name=f"dsum{b}_{qt}_{h}")
                nc.tensor.matmul(dsum, ones_sb, fold, start=True,
                                 stop=True, skip_group_check=True)
                # single-op DVE Newton reciprocal straight off PSUM (no
                # DMA-spread roundtrip); 18 correct bits, plenty for bf16
                rec = small.tile([1, RT], f32, tag="rec")
                nc.vector.reciprocal_approx_fast(out=rec, in_=dsum)
                rec_bf = small.tile([1, RT], bf16, tag="recb")
                nc.scalar.activation(rec_bf, rec, Copy)
                bc = small.tile([128, RT], bf16, tag="bc")
                nc.gpsimd.partition_broadcast(out_ap=bc, in_ap=rec_bf)
                if fused:
                    nc.vector.tensor_mul(yT_sb[b][:, h, qsl], y_ps[h], bc)
                else:
                    nc.vector.tensor_mul(yT_sb[b][:, h, qsl],
                                         yT_sb[b][:, h, qsl], bc)

            def attn_qt(b, qt, interleave=None, defer_last_fin=False,
                        fused_tail=False):
                """all attention work for one query tile; interleave is a
                list of thunks emitted between key-block units.

                h-outer: head 0 finishes halfway through the tile so its
                finalize chain (reciprocal etc.) hides under head 1's
                attention units instead of stalling the next tile.  The
                last head's finalize can be deferred (returned as a thunk
                for the caller to weave into the NEXT tile) so its dsum
                matmul never head-of-line blocks the in-order PE queue
                while the DVE fold catches up."""
                y_ps = [psy.tile([d, RT], f32, tag=f"y{h}",
                                 name=f"y{b}_{qt}_{h}")
                        for h in range(hpc)]
                daccs = [dpool.tile([128, 4, RT], bf16, tag="dacc",
                                    name=f"dacc{b}_{qt}_{h}")
                         for h in range(hpc)]
                equads = [None] * hpc
                pend = [[] for _ in range(hpc)]
                il = list(interleave or [])
                # spread the filler thunks across the unit sequence
                nu = hpc * n_kb
                points = {}
                for i, th in enumerate(il):
                    points.setdefault(min(nu - 1, i * nu // len(il)),
                                      []).append(th)
                u = 0
                for h in range(hpc):
                    for kb in range(n_kb):
                        attn_unit(b, qt, kb, h, y_ps, daccs, equads, pend)
                        # lag the MM2s two key-blocks behind their exp so
                        # the PE never head-of-line blocks on ScalarE
                        mm2_drain(b, h, y_ps, pend, 2)
                        for th in points.get(u, []):
                            th()
                        u += 1
                    mm2_drain(b, h, y_ps, pend, 0)
                    if h < hpc - 1:
                        finalize(b, qt, h, y_ps, daccs)
                hl = hpc - 1
                fin = (lambda: finalize(b, qt, hl, y_ps, daccs,
                                        fused=fused_tail))
                if defer_last_fin:
                    return fin
                fin()
                return None

            def outproj_unit(b, rb, ot, eng):
                o_ps = pso.tile([128, RT], f32, tag="o")
                for h in range(hpc):
                    nc.tensor.matmul(
                        o_ps,
                        yT_sb[b][:, h, rb * 128:(rb + 1) * 128],
                        wp_sb[:, h, ot * RT:(ot + 1) * RT],
                        start=(h == 0), stop=(h == hpc - 1),
                        skip_group_check=True)
                # cast on DVE/ACT (gpsimd has no PSUM port); the DIRECT2D
                # dispatch rides the sync queue -- idle in P3 (the x
                # stream is done) -- so exp ACTs and broadcasts never
                # queue behind store dispatch
                o_sb = opool.tile([128, RT], bf16, tag="o")
                if eng == 0:
                    nc.vector.tensor_copy(out=o_sb, in_=o_ps)
                else:
                    nc.scalar.activation(o_sb, o_ps, Copy)
                nc.sync.dma_start(
                    out[b * Tp + rb * 128:b * Tp + (rb + 1) * 128,
                        ot * RT:(ot + 1) * RT],
                    o_sb)

            # ================= emission schedule =================
            ppb = n_rt // n_qt          # proj tiles per query tile
            opb = n_rb // n_qt          # row blocks per query tile
            cnt = [0]
            # P1: proj b0 with attn(b0, qt0) streaming kb-wise behind
            # the tiles that produce its k/v blocks (fills the otherwise
            # idle ScalarE and covers DMA stalls with PE work)
            yq0 = [psy.tile([d, RT], f32, tag=f"y{h}", name=f"yq0_{h}")
                   for h in range(hpc)]
            dq0 = [dpool.tile([128, 4, RT], bf16, tag="dacc",
                              name=f"daccq0_{h}") for h in range(hpc)]
            eq0 = [None] * hpc
            pq0 = [[] for _ in range(hpc)]
            for rt in range(n_rt):
                if rt == 0:
                    # progressive ck-parts: the first matmuls wait only on
                    # the first one or two w/x chunk DMAs
                    st0 = {}
                    for lo, hi in ((0, 1), (1, 2), (2, 4), (4, 8), (8, 16)):
                        proj_tile_range(0, 0, lo, hi, st0)
                else:
                    proj_tile(0, rt)
                if rt == 1:
                    # wp is first used in P3; deferring its 1MB DMA keeps
                    # early HBM bandwidth for the w/x critical path
                    nc.scalar.dma_start(
                        wp_sb, wp.rearrange("(h p) o -> p h o", p=128))
                if rt >= 2:
                    for kb in (2 * (rt - 2), 2 * (rt - 2) + 1):
                        for h in range(hpc):
                            attn_unit(0, 0, kb, h, yq0, dq0, eq0, pq0)
                            mm2_drain(0, h, yq0, pq0, 2)
            # finish qt0's remaining key blocks, then finalize it
            for kb in range(2 * (n_rt - 2), n_kb):
                for h in range(hpc):
                    attn_unit(0, 0, kb, h, yq0, dq0, eq0, pq0)
                    mm2_drain(0, h, yq0, pq0, 2)
            for h in range(hpc):
                mm2_drain(0, h, yq0, pq0, 0)
            for h in range(hpc - 1):
                finalize(0, 0, h, yq0, dq0)
            pend_fin = lambda: finalize(0, 0, hpc - 1, yq0, dq0)
            # P2: attn b0 qt1-3 interleaved with ALL 8 proj-b1 tiles
            # (quarter-tile weave, distributed across the 3 query tiles)
            # plus the previous tile's b0 outproj rows: P2 is PE-bound
            # with DVE slack, so the store casts are free here, and P3
            # (where DVE is co-critical) keeps only the b1 stores.
            # The previous tile's deferred finalize leads each weave.
            def op_thunks(units):
                ths = []
                for i in range(0, len(units), 2):
                    chunk = units[i:i + 2]
                    def th(chunk=chunk):
                        for b_, rb, ot in chunk:
                            cnt[0] += 1
                            outproj_unit(b_, rb, ot, 0)
                    ths.append(th)
                return ths
            noop = lambda: None
            def mix(a, bl):
                out, ia, ib = [], 0, 0
                while ia < len(a) or ib < len(bl):
                    if ia * (len(bl) + 1) <= ib * (len(a) + 1):
                        if ia < len(a):
                            out.append(a[ia])
                        ia += 1
                    else:
                        if ib < len(bl):
                            out.append(bl[ib])
                        ib += 1
                return out
            for qt in range(1, n_qt):
                j = qt - 1
                pthunks = []
                for rt in range(j * n_rt // 3, (j + 1) * n_rt // 3):
                    st = {}
                    for part in range(4):
                        pthunks.append(
                            lambda rt=rt, part=part, st=st:
                                proj_tile_range(1, rt, part * 4,
                                                (part + 1) * 4, st))
                thunks = [pend_fin] + pthunks
                pend_fin = attn_qt(0, qt, interleave=thunks,
                                   defer_last_fin=True)
            # proj + qkv PSUM done -> release banks for the P3 pool
            psoA_cm.__exit__(None, None, None)
            psp_cm.__exit__(None, None, None)
            psoB_cm = tc.tile_pool(name="psoB", bufs=3, space="PSUM")
            pso = psoB_cm.__enter__()
            # P3: attn b1; outproj woven between units: b0 rows early
            # (long finalized), the previous tile's b1 rows late
            for qt in range(n_qt):
                units = [(0, rb, ot)
                         for rb in range(qt * opb, (qt + 1) * opb)
                         for ot in range(n_ot)]
                units1 = [(1, rb, ot)
                          for rb in range((qt - 1) * opb, qt * opb)
                          for ot in range(n_ot)] if qt > 0 else []
                thunks = ([pend_fin, noop] + op_thunks(units)
                          + op_thunks(units1))
                last = qt == n_qt - 1
                pend_fin = attn_qt(1, qt, interleave=thunks,
                                   defer_last_fin=not last,
                                   fused_tail=last)
            # P4: last query tile's outproj b1; alternate cast engines
            # (ACT is idle here) so the drain is not CAST-bound
            for rb in range((n_qt - 1) * opb, n_qt * opb):
                for ot in range(n_ot):
                    cnt[0] += 1
                    outproj_unit(1, rb, ot, ot % 2)
            psoB_cm.__exit__(None, None, None)

    nc.compile()
    return nc


def _prep_in_maps(x, cos, sin, W_qkv, W_proj, n_cores, hpc, d):
    """Host-side shard prep: pure layout work (transpose / slice / sign fold)."""
    Bp, Tp, Cp = x.shape
    jc = hpc * d
    import ml_dtypes
    tdt = ml_dtypes.bfloat16 if TABLE_DT == "bf16" else np.float32
    xTa = np.ascontiguousarray(x.reshape(Bp * Tp, Cp).T).astype(ml_dtypes.bfloat16)
    cosT = np.ascontiguousarray(cos.T).astype(tdt)
    sinT = np.ascontiguousarray(sin.T).copy()
    sinT[: d // 2] *= -1.0
    sinT = sinT.astype(tdt)
    in_maps = []
    for c in range(n_cores):
        j0, j1 = c * jc, (c + 1) * jc
        in_maps.append({
            "xT": xTa,
            "wqkv": np.ascontiguousarray(np.concatenate(
                [W_qkv[:, j0:j1], W_qkv[:, Cp + j0:Cp + j1],
                 W_qkv[:, 2 * Cp + j0:2 * Cp + j1]], axis=1,
            )).astype(ml_dtypes.bfloat16),
            "wp": np.ascontiguousarray(W_proj[j0:j1, :]).astype(ml_dtypes.bfloat16),
            "ones": np.ones((128, 1), dtype=ml_dtypes.bfloat16),
            "cosT": cosT,
            "sinT": sinT,
        })
    return in_maps


def _install_ntff_hook():
    """Enable NTFF profiling under axon when the boot image lacks the
    antenv.axon_hooks shim. Harmless if anything is missing."""
    import sys
    import types
    try:
        from antenv.axon_hooks import get_axon_ntff_profile_hook
        if get_axon_ntff_profile_hook() is not None:
            return
    except ImportError:
        pass
    try:
        sys.path.insert(0, "/root/.axon_site")
        from trn_agent_boot.trn_boot import _ntff_profile_via_ctypes

        hook = _ntff_profile_via_ctypes("/opt/axon/libaxon_pjrt.so")
        if hook is None:
            return
        mod = types.ModuleType("antenv.axon_hooks")
        mod.get_axon_ntff_profile_hook = lambda: hook
        mod.set_axon_ntff_profile_hook = lambda h: None
        import antenv
        antenv.axon_hooks = mod
        sys.modules["antenv.axon_hooks"] = mod
    except Exception:
        pass


def _run(x, cos, sin, W_qkv, W_proj, trace=False):
    from concourse.bass_utils import run_bass_kernel_spmd

    if trace:
        _install_ntff_hook()

    x = np.ascontiguousarray(x, dtype=np.float32)
    cos = np.ascontiguousarray(cos, dtype=np.float32)
    sin = np.ascontiguousarray(sin, dtype=np.float32)
    W_qkv = np.ascontiguousarray(W_qkv, dtype=np.float32)
    W_proj = np.ascontiguousarray(W_proj, dtype=np.float32)

    Bp, Tp, Cp = x.shape
    nc = _build(Bp, Tp, Cp, HPC, D)
    in_maps = _prep_in_maps(x, cos, sin, W_qkv, W_proj, N_CORES, HPC, D)
    res = run_bass_kernel_spmd(nc, in_maps, core_ids=list(range(N_CORES)),
                               trace=trace)
    acc = np.zeros((Bp * Tp, Cp), dtype=np.float32)
    for i in range(N_CORES):
        acc += np.asarray(res.results[i]["out"], dtype=np.float32)
    return acc.reshape(Bp, Tp, Cp), res


def kernel(x, cos, sin, W_qkv, W_proj):
    out, _ = _run(x, cos, sin, W_qkv, W_proj, trace=False)
    return out



# revision 48
# speedup vs baseline: 1.8084x; 1.0071x over previous
"""Trainium2 Bass kernel v2: multi-head attention (B=2, T=2048, C=2048, H=16, D=128).

Sharding: tensor-parallel over heads. 8 cores x 2 heads each.
  - W_qkv columns sliced per head-pair, W_proj rows sliced per head-pair.
  - Each core computes a partial output [B*T, C]; host sums the 8 partials.

v3 changes vs v2 (435us):
  - out DMA'd straight from PSUM (f32) on the gpsimd queue: kills 256
    out-casts (~170us DVE) and moves DIRECT2D dispatch off the Scalar
    queue so exp ACTs never queue behind store dispatch.
  - finalize decoupled: unnormalized y copied out of PSUM immediately
    (ACT), reciprocal chain runs off the critical path, normalize is a
    late in-place bf16 DVE mul.  Next qt's MM2s no longer wait on the
    reciprocal DMA roundtrip.
  - dacc quads folded 4->1 on DVE before the ones-matmul: denominator
    contraction is 512 PE rows instead of 2048 (-10us PE).
  - bf16 rope tables (halves table DMA), wp DMA deferred past P1.
  - progressive ck-parts for the first proj tile so the first matmul
    only waits on the first w/x chunk DMAs, not 8 of them.
"""

import math

import numpy as np

N_CORES = 8
B, T, C = 2, 2048, 2048
N_HEAD, D = 16, 128
HPC = N_HEAD // N_CORES          # heads per core
JC = HPC * D                     # per-core slice width of qkv/proj dims

RP = 256                         # proj token tile (moving free dim)
RT = 512                         # attention query tile
KB = 128                         # key block (contraction tile)

# rope table dtype: "bf16" saves SBUF + DMA, needs mixed-dtype TT support
TABLE_DT = "bf16"


def _build(Bp, Tp, Cp, hpc, d):
    import concourse.bacc as bacc
    import concourse.tile as tile
    from concourse import mybir

    f32 = mybir.dt.float32
    bf16 = mybir.dt.bfloat16
    Exp = mybir.ActivationFunctionType.Exp
    Ln = mybir.ActivationFunctionType.Ln
    Copy = mybir.ActivationFunctionType.Copy

    jc = hpc * d
    BT = Bp * Tp
    n_ck = Cp // 128             # contraction chunks for proj
    n_rt = Tp // RP              # proj token tiles per batch
    n_sub = RP // 128            # v sub-blocks per proj tile
    n_kb = Tp // KB              # key blocks per batch
    n_kbp = n_kb // 2            # key-block pairs
    n_qt = Tp // RT              # query tiles per batch
    n_rb = Tp // 128             # row blocks for out proj
    n_ot = Cp // RT              # output column tiles
    scale = 1.0 / math.sqrt(d)
    hd = d // 2
    tdt = bf16 if TABLE_DT == "bf16" else f32

    nc = bacc.Bacc("TRN2", target_bir_lowering=False, debug=False)

    xT = nc.declare_dram_parameter("xT", [Cp, BT], bf16, isOutput=False)
    wqkv = nc.declare_dram_parameter("wqkv", [Cp, 3 * jc], bf16,
                                     isOutput=False)
    wp = nc.declare_dram_parameter("wp", [jc, Cp], bf16, isOutput=False)
    ones_d = nc.declare_dram_parameter("ones", [128, 1], bf16, isOutput=False)
    cosT = nc.declare_dram_parameter("cosT", [d, Tp], tdt, isOutput=False)
    sinT = nc.declare_dram_parameter("sinT", [d, Tp], tdt, isOutput=False)
    out = nc.declare_dram_parameter("out", [BT, Cp], bf16, isOutput=True)

    with tile.TileContext(nc) as tc:
        with (
            nc.allow_low_precision(reason="bf16 staging, f32 PSUM accum"),
            tc.tile_pool(name="wpool", bufs=1) as wpool,
            tc.tile_pool(name="acts", bufs=1) as acts,
            tc.tile_pool(name="xpool", bufs=17) as xpool,
            tc.tile_pool(name="rope", bufs=4) as ropep,
            tc.tile_pool(name="epool", bufs=4) as epool,
            tc.tile_pool(name="dpool", bufs=3) as dpool,
            tc.tile_pool(name="small", bufs=2) as small,
            tc.tile_pool(name="opool", bufs=6) as opool,
            tc.tile_pool(name="pss", bufs=2, space="PSUM") as pss,
            tc.tile_pool(name="psy", bufs=1, space="PSUM") as psy,
        ):
            psp_cm = tc.tile_pool(name="psp", bufs=1, space="PSUM")
            psp = psp_cm.__enter__()
            psoA_cm = tc.tile_pool(name="psoA", bufs=1, space="PSUM")
            pso = psoA_cm.__enter__()
            # ---- resident weights / tables ----
            # inputs (w, x) stream on the sync/SP queue; tables + wp + all
            # output traffic go on the scalar queue so they never starve
            # the x-tile stream.
            wq_sb, wk_sb, wv_sb = [], [], []
            xpair = {}
            for ck in range(n_ck):
                # x before w per ck so the first matmul's sem wait covers
                # the fewest preceding queue entries; rt0-only tiles so
                # the wait prefix is 260KB per ck, not 324KB+
                xp = xpool.tile([128, RP], bf16, tag="xs",
                                name=f"xtpre{ck}")
                nc.sync.dma_start(xp, xT[ck * 128:(ck + 1) * 128, 0:RP])
                xpair[("s", 0, ck)] = xp
                t = wpool.tile([128, 3 * jc], bf16, tag=f"w{ck}",
                               name=f"w{ck}")
                nc.sync.dma_start(t, wqkv[ck * 128:(ck + 1) * 128, :])
                wq_sb.append(t[:, 0:jc])
                wk_sb.append(t[:, jc:2 * jc])
                wv_sb.append(t[:, 2 * jc:3 * jc])
            ones_sb = wpool.tile([128, 1], bf16, tag="ones")
            nc.sync.dma_start(ones_sb, ones_d[:])
            # doubled rope tables for h-batched rope: [d, hpc, Tp]
            cos2 = wpool.tile([d, hpc, Tp], tdt, tag="cos2")
            sin2 = wpool.tile([d, hpc, Tp], tdt, tag="sin2")
            for h in range(hpc):
                nc.scalar.dma_start(cos2[:, h, :], cosT[:])
                nc.scalar.dma_start(sin2[:, h, :], sinT[:])
            wp_sb = wpool.tile([128, hpc, Cp], bf16, tag="wp")

            # per-batch activation tensors (both batches resident -> the
            # scheduler can overlap attn(b) with proj(b+1))
            qT_sb = [acts.tile([128, hpc, Tp], bf16, tag=f"qT{b}",
                               name=f"qT{b}") for b in range(Bp)]
            kT_sb = [acts.tile([128, hpc, Tp], bf16, tag=f"kT{b}",
                               name=f"kT{b}") for b in range(Bp)]
            v_sb = [acts.tile([128, n_kb, jc], bf16, tag=f"v{b}",
                              name=f"v{b}") for b in range(Bp)]
            yT_sb = [acts.tile([128, hpc, Tp], bf16, tag=f"yT{b}",
                               name=f"yT{b}") for b in range(Bp)]

            def proj_tile_range(b, rt, ck_lo, ck_hi, state):
                """qkv projection for one RP-wide token tile + rope,
                emitted in ck-range parts so attention work can weave
                between them at fine grain."""
                tsl = slice(rt * RP, (rt + 1) * RP)
                if ck_lo == 0:
                    state["q"] = psp.tile([128, hpc, RP], f32, tag="qps",
                                          name=f"qps{b}_{rt}")
                    state["k"] = psp.tile([128, hpc, RP], f32, tag="kps",
                                          name=f"kps{b}_{rt}")
                    state["v"] = psp.tile([128, n_sub, jc], f32, tag="vps",
                                          name=f"vps{b}_{rt}")
                q_ps, k_ps, v_ps = state["q"], state["k"], state["v"]
                for ck in range(ck_lo, ck_hi):
                    if b == 0 and rt < 2:
                        # first two token tiles: per-rt x tiles so each
                        # proj part waits on the minimum queue prefix
                        # during the DMA-limited startup
                        pk = ("s", rt, ck)
                        if pk not in xpair:
                            xp = xpool.tile([128, RP], bf16, tag="xs")
                            p0 = rt * RP
                            nc.sync.dma_start(
                                xp, xT[ck * 128:(ck + 1) * 128,
                                       p0:p0 + RP])
                            xpair[pk] = xp
                        xt = xpair[pk]
                    else:
                        # x streamed in [128, 2*RP] pair-slabs
                        pk = (b, rt // 2, ck)
                        if pk not in xpair:
                            xp = xpool.tile([128, 2 * RP], bf16, tag="xt")
                            p0 = b * Tp + (rt // 2) * 2 * RP
                            nc.sync.dma_start(
                                xp, xT[ck * 128:(ck + 1) * 128,
                                       p0:p0 + 2 * RP])
                            xpair[pk] = xp
                        xt = xpair[pk][:, (rt % 2) * RP:(rt % 2) * RP + RP]
                    first = ck == 0
                    last = ck == n_ck - 1
                    for h in range(hpc):
                        nc.tensor.matmul(
                            q_ps[:, h, :],
                            wq_sb[ck][:, h * d:(h + 1) * d],
                            xt, start=(first and h == 0),
                            stop=(last and h == hpc - 1),
                            skip_group_check=True)
                        nc.tensor.matmul(
                            k_ps[:, h, :],
                            wk_sb[ck][:, h * d:(h + 1) * d],
                            xt, start=(first and h == 0),
                            stop=(last and h == hpc - 1),
                            skip_group_check=True)
                    for s in range(n_sub):
                        nc.tensor.matmul(
                            v_ps[:, s, :],
                            xt[:, s * 128:(s + 1) * 128],
                            wv_sb[ck], start=(first and s == 0),
                            stop=(last and s == n_sub - 1),
                            skip_group_check=True)
                if ck_hi != n_ck:
                    return
                # h-batched rope epilogue on [128, hpc*RP]:
                #   dst = psum*cos2 + swap(psum)*sin2_signed
                for ps, dst in ((q_ps, qT_sb[b]), (k_ps, kT_sb[b])):
                    c2 = cos2[:, :, tsl]
                    s2 = sin2[:, :, tsl]
                    t1 = ropep.tile([d, hpc, RP], f32, tag="t1")
                    nc.vector.tensor_mul(t1, ps, c2)
                    t2 = ropep.tile([d, hpc, RP], f32, tag="t2")
                    nc.vector.tensor_mul(t2[0:hd], ps[hd:d], s2[0:hd])
                    nc.vector.tensor_mul(t2[hd:d], ps[0:hd], s2[hd:d])
                    nc.vector.tensor_add(dst[:, :, tsl], t1, t2)
                # v copy (both sub-blocks in one ACT instruction)
                nc.scalar.activation(
                    v_sb[b][:, rt * n_sub:(rt + 1) * n_sub, :], v_ps, Copy)

            def proj_tile(b, rt):
                st = {}
                for part in range(4):
                    proj_tile_range(b, rt, part * 4, (part + 1) * 4, st)

            def attn_unit(b, qt, kb, h, y_ps, daccs, equads, pend):
                """one (query-tile, key-block, head) attention step.

                s tiles are single-bank [128, RT], double-buffered -> the
                next MM1 overlaps the current exp.  e tiles are quads
                [128, 4, RT] shared by 4 consecutive key blocks so dacc
                accumulates 2048 elems per DVE op.
                """
                qsl = slice(qt * RT, (qt + 1) * RT)
                s_ps = pss.tile([128, RT], f32, tag="s")
                nc.tensor.matmul(
                    s_ps,
                    kT_sb[b][:, h, kb * KB:(kb + 1) * KB],
                    qT_sb[b][:, h, qsl],
                    start=True, stop=True, skip_group_check=True)
                if kb % 4 == 0:
                    equads[h] = epool.tile([128, 4, RT], bf16, tag="e",
                                           name=f"e{b}_{qt}_{h}_{kb}")
                eq = equads[h]
                nc.scalar.activation(eq[:, kb % 4, :], s_ps, Exp,
                                     scale=scale)
                if kb % 4 == 3:
                    qd = kb // 4
                    if qd == 0:
                        nc.vector.tensor_copy(out=daccs[h], in_=eq)
                    else:
                        nc.vector.tensor_add(daccs[h], daccs[h], eq)
                pend[h].append((kb, eq))

            def mm2_drain(b, h, y_ps, pend, keep):
                while len(pend[h]) > keep:
                    kb, eq = pend[h].pop(0)
                    nc.tensor.matmul(
                        y_ps[h],
                        v_sb[b][:, kb, h * d:(h + 1) * d],
                        eq[:, kb % 4, :],
                        start=(kb == 0), stop=(kb == n_kb - 1),
                        skip_group_check=True)

            def finalize(b, qt, h, y_ps, daccs, fused=False):
                qsl = slice(qt * RT, (qt + 1) * RT)
                # free the PSUM bank immediately: unnormalized copy; the
                # reciprocal chain below runs off the critical path and
                # the normalize is a late in-place bf16 mul.  (fused path
                # for the very last tile: keep y in PSUM, one less link.)
                if not fused:
                    nc.scalar.activation(yT_sb[b][:, h, qsl], y_ps[h], Copy)
                # fold dacc quads 4->1 so the ones-matmul is 512 rows
                f2 = small.tile([128, 2, RT], bf16, tag="f2")
                nc.vector.tensor_add(f2, daccs[h][:, 0:2, :],
                                     daccs[h][:, 2:4, :])
                fold = small.tile([128, RT], bf16, tag="fold")
                nc.vector.tensor_add(fold, f2[:, 0, :], f2[:, 1, :])
                dsum = pso.tile([1, RT], f32, tag="fin", bufs=1,
                                name=f"dsum{b}_{qt}_{h}")
                nc.tensor.matmul(dsum, ones_sb, fold, start=True,
                                 stop=True, skip_group_check=True)
                # single-op DVE Newton reciprocal straight off PSUM (no
                # DMA-spread roundtrip); 18 correct bits, plenty for bf16
                rec = small.tile([1, RT], f32, tag="rec")
                nc.vector.reciprocal_approx_fast(out=rec, in_=dsum)
                if fused:
                    # tail path: broadcast f32 directly (skip the bf16
                    # recast hop) and multiply straight out of PSUM
                    bcf = small.tile([128, RT], f32, tag="bcf")
                    nc.gpsimd.partition_broadcast(out_ap=bcf, in_ap=rec)
                    nc.vector.tensor_mul(yT_sb[b][:, h, qsl], y_ps[h],
                                         bcf)
                else:
                    rec_bf = small.tile([1, RT], bf16, tag="recb")
                    nc.scalar.activation(rec_bf, rec, Copy)
                    bc = small.tile([128, RT], bf16, tag="bc")
                    nc.gpsimd.partition_broadcast(out_ap=bc, in_ap=rec_bf)
                    nc.vector.tensor_mul(yT_sb[b][:, h, qsl],
                                         yT_sb[b][:, h, qsl], bc)

            def attn_qt(b, qt, interleave=None, defer_last_fin=False,
                        fused_tail=False):
                """all attention work for one query tile; interleave is a
                list of thunks emitted between key-block units.

                h-outer: head 0 finishes halfway through the tile so its
                finalize chain (reciprocal etc.) hides under head 1's
                attention units instead of stalling the next tile.  The
                last head's finalize can be deferred (returned as a thunk
                for the caller to weave into the NEXT tile) so its dsum
                matmul never head-of-line blocks the in-order PE queue
                while the DVE fold catches up."""
                y_ps = [psy.tile([d, RT], f32, tag=f"y{h}",
                                 name=f"y{b}_{qt}_{h}")
                        for h in range(hpc)]
                daccs = [dpool.tile([128, 4, RT], bf16, tag="dacc",
                                    name=f"dacc{b}_{qt}_{h}")
                         for h in range(hpc)]
                equads = [None] * hpc
                pend = [[] for _ in range(hpc)]
                il = list(interleave or [])
                # spread the filler thunks across the unit sequence
                nu = hpc * n_kb
                points = {}
                for i, th in enumerate(il):
                    points.setdefault(min(nu - 1, i * nu // len(il)),
                                      []).append(th)
                u = 0
                for h in range(hpc):
                    for kb in range(n_kb):
                        attn_unit(b, qt, kb, h, y_ps, daccs, equads, pend)
                        # lag the MM2s two key-blocks behind their exp so
                        # the PE never head-of-line blocks on ScalarE
                        mm2_drain(b, h, y_ps, pend, 2)
                        for th in points.get(u, []):
                            th()
                        u += 1
                    mm2_drain(b, h, y_ps, pend, 0)
                    if h < hpc - 1:
                        finalize(b, qt, h, y_ps, daccs)
                hl = hpc - 1
                fin = (lambda: finalize(b, qt, hl, y_ps, daccs,
                                        fused=fused_tail))
                if defer_last_fin:
                    return fin
                fin()
                return None

            def outproj_unit(b, rb, ot, eng):
                o_ps = pso.tile([128, RT], f32, tag="o")
                for h in range(hpc):
                    nc.tensor.matmul(
                        o_ps,
                        yT_sb[b][:, h, rb * 128:(rb + 1) * 128],
                        wp_sb[:, h, ot * RT:(ot + 1) * RT],
                        start=(h == 0), stop=(h == hpc - 1),
                        skip_group_check=True)
                # cast on DVE/ACT (gpsimd has no PSUM port); the DIRECT2D
                # dispatch rides the sync queue -- idle in P3 (the x
                # stream is done) -- so exp ACTs and broadcasts never
                # queue behind store dispatch
                o_sb = opool.tile([128, RT], bf16, tag="o")
                if eng == 0:
                    nc.vector.tensor_copy(out=o_sb, in_=o_ps)
                else:
                    nc.scalar.activation(o_sb, o_ps, Copy)
                nc.sync.dma_start(
                    out[b * Tp + rb * 128:b * Tp + (rb + 1) * 128,
                        ot * RT:(ot + 1) * RT],
                    o_sb)

            # ================= emission schedule =================
            ppb = n_rt // n_qt          # proj tiles per query tile
            opb = n_rb // n_qt          # row blocks per query tile
            cnt = [0]
            # P1: proj b0 with attn(b0, qt0) streaming kb-wise behind
            # the tiles that produce its k/v blocks (fills the otherwise
            # idle ScalarE and covers DMA stalls with PE work)
            yq0 = [psy.tile([d, RT], f32, tag=f"y{h}", name=f"yq0_{h}")
                   for h in range(hpc)]
            dq0 = [dpool.tile([128, 4, RT], bf16, tag="dacc",
                              name=f"daccq0_{h}") for h in range(hpc)]
            eq0 = [None] * hpc
            pq0 = [[] for _ in range(hpc)]
            for rt in range(n_rt):
                if rt == 0:
                    # progressive ck-parts: the first matmuls wait only on
                    # the first one or two w/x chunk DMAs
                    st0 = {}
                    for lo, hi in ((0, 1), (1, 2), (2, 4), (4, 8), (8, 16)):
                        proj_tile_range(0, 0, lo, hi, st0)
                else:
                    proj_tile(0, rt)
                if rt == 1:
                    # wp is first used in P3; deferring its 1MB DMA keeps
                    # early HBM bandwidth for the w/x critical path
                    nc.scalar.dma_start(
                        wp_sb, wp.rearrange("(h p) o -> p h o", p=128))
                if rt >= 2:
                    for kb in (2 * (rt - 2), 2 * (rt - 2) + 1):
                        for h in range(hpc):
                            attn_unit(0, 0, kb, h, yq0, dq0, eq0, pq0)
                            mm2_drain(0, h, yq0, pq0, 2)
            # finish qt0's remaining key blocks, then finalize it
            for kb in range(2 * (n_rt - 2), n_kb):
                for h in range(hpc):
                    attn_unit(0, 0, kb, h, yq0, dq0, eq0, pq0)
                    mm2_drain(0, h, yq0, pq0, 2)
            for h in range(hpc):
                mm2_drain(0, h, yq0, pq0, 0)
            for h in range(hpc - 1):
                finalize(0, 0, h, yq0, dq0)
            pend_fin = lambda: finalize(0, 0, hpc - 1, yq0, dq0)
            # P2: attn b0 qt1-3 interleaved with ALL 8 proj-b1 tiles
            # (quarter-tile weave, distributed across the 3 query tiles)
            # plus the previous tile's b0 outproj rows: P2 is PE-bound
            # with DVE slack, so the store casts are free here, and P3
            # (where DVE is co-critical) keeps only the b1 stores.
            # The previous tile's deferred finalize leads each weave.
            def op_thunks(units, alt=False):
                ths = []
                for i in range(0, len(units), 2):
                    chunk = units[i:i + 2]
                    def th(chunk=chunk):
                        for j, (b_, rb, ot) in enumerate(chunk):
                            cnt[0] += 1
                            outproj_unit(b_, rb, ot,
                                         (ot % 2) if alt else 0)
                    ths.append(th)
                return ths
            noop = lambda: None
            def mix(a, bl):
                out, ia, ib = [], 0, 0
                while ia < len(a) or ib < len(bl):
                    if ia * (len(bl) + 1) <= ib * (len(a) + 1):
                        if ia < len(a):
                            out.append(a[ia])
                        ia += 1
                    else:
                        if ib < len(bl):
                            out.append(bl[ib])
                        ib += 1
                return out
            for qt in range(1, n_qt):
                j = qt - 1
                pthunks = []
                for rt in range(j * n_rt // 3, (j + 1) * n_rt // 3):
                    st = {}
                    for part in range(4):
                        pthunks.append(
                            lambda rt=rt, part=part, st=st:
                                proj_tile_range(1, rt, part * 4,
                                                (part + 1) * 4, st))
                thunks = [pend_fin] + pthunks
                pend_fin = attn_qt(0, qt, interleave=thunks,
                                   defer_last_fin=True)
            # proj + qkv PSUM done -> release banks for the P3 pool
            psoA_cm.__exit__(None, None, None)
            psp_cm.__exit__(None, None, None)
            psoB_cm = tc.tile_pool(name="psoB", bufs=3, space="PSUM")
            pso = psoB_cm.__enter__()
            # P3: attn b1; outproj woven between units: b0 rows early
            # (long finalized), the previous tile's b1 rows late
            for qt in range(n_qt):
                units = [(0, rb, ot)
                         for rb in range(qt * opb, (qt + 1) * opb)
                         for ot in range(n_ot)]
                units1 = [(1, rb, ot)
                          for rb in range((qt - 1) * opb, qt * opb)
                          for ot in range(n_ot)] if qt > 0 else []
                last = qt == n_qt - 1
                # on the last tile the late stores' casts alternate onto
                # ACT (its exps are done by then) so the DVE queue can
                # reach the tail finalize's folds sooner
                thunks = ([pend_fin, noop] + op_thunks(units)
                          + op_thunks(units1, alt=last))
                pend_fin = attn_qt(1, qt, interleave=thunks,
                                   defer_last_fin=not last,
                                   fused_tail=last)
            # P4: last query tile's outproj b1; alternate cast engines
            # (ACT is idle here) so the drain is not CAST-bound
            for rb in range((n_qt - 1) * opb, n_qt * opb):
                for ot in range(n_ot):
                    cnt[0] += 1
                    outproj_unit(1, rb, ot, ot % 2)
            psoB_cm.__exit__(None, None, None)

    nc.compile()
    return nc


def _prep_in_maps(x, cos, sin, W_qkv, W_proj, n_cores, hpc, d):
    """Host-side shard prep: pure layout work (transpose / slice / sign fold)."""
    Bp, Tp, Cp = x.shape
    jc = hpc * d
    import ml_dtypes
    tdt = ml_dtypes.bfloat16 if TABLE_DT == "bf16" else np.float32
    xTa = np.ascontiguousarray(x.reshape(Bp * Tp, Cp).T).astype(ml_dtypes.bfloat16)
    cosT = np.ascontiguousarray(cos.T).astype(tdt)
    sinT = np.ascontiguousarray(sin.T).copy()
    sinT[: d // 2] *= -1.0
    sinT = sinT.astype(tdt)
    in_maps = []
    for c in range(n_cores):
        j0, j1 = c * jc, (c + 1) * jc
        in_maps.append({
            "xT": xTa,
            "wqkv": np.ascontiguousarray(np.concatenate(
                [W_qkv[:, j0:j1], W_qkv[:, Cp + j0:Cp + j1],
                 W_qkv[:, 2 * Cp + j0:2 * Cp + j1]], axis=1,
            )).astype(ml_dtypes.bfloat16),
            "wp": np.ascontiguousarray(W_proj[j0:j1, :]).astype(ml_dtypes.bfloat16),
            "ones": np.ones((128, 1), dtype=ml_dtypes.bfloat16),
            "cosT": cosT,
            "sinT": sinT,
        })
    return in_maps


def _install_ntff_hook():
    """Enable NTFF profiling under axon when the boot image lacks the
    antenv.axon_hooks shim. Harmless if anything is missing."""
    import sys
    import types
    try:
        from antenv.axon_hooks import get_axon_ntff_profile_hook
        if get_axon_ntff_profile_hook() is not None:
            return
    except ImportError:
        pass
    try:
        sys.path.insert(0, "/root/.axon_site")
        from trn_agent_boot.trn_boot import _ntff_profile_via_ctypes

        hook = _ntff_profile_via_ctypes("/opt/axon/libaxon_pjrt.so")
        if hook is None:
            return
        mod = types.ModuleType("antenv.axon_hooks")
        mod.get_axon_ntff_profile_hook = lambda: hook
        mod.set_axon_ntff_profile_hook = lambda h: None
        import antenv
        antenv.axon_hooks = mod
        sys.modules["antenv.axon_hooks"] = mod
    except Exception:
        pass


def _run(x, cos, sin, W_qkv, W_proj, trace=False):
    from concourse.bass_utils import run_bass_kernel_spmd

    if trace:
        _install_ntff_hook()

    x = np.ascontiguousarray(x, dtype=np.float32)
    cos = np.ascontiguousarray(cos, dtype=np.float32)
    sin = np.ascontiguousarray(sin, dtype=np.float32)
    W_qkv = np.ascontiguousarray(W_qkv, dtype=np.float32)
    W_proj = np.ascontiguousarray(W_proj, dtype=np.float32)

    Bp, Tp, Cp = x.shape
    nc = _build(Bp, Tp, Cp, HPC, D)
    in_maps = _prep_in_maps(x, cos, sin, W_qkv, W_proj, N_CORES, HPC, D)
    res = run_bass_kernel_spmd(nc, in_maps, core_ids=list(range(N_CORES)),
                               trace=trace)
    acc = np.zeros((Bp * Tp, Cp), dtype=np.float32)
    for i in range(N_CORES):
        acc += np.asarray(res.results[i]["out"], dtype=np.float32)
    return acc.reshape(Bp, Tp, Cp), res


def kernel(x, cos, sin, W_qkv, W_proj):
    out, _ = _run(x, cos, sin, W_qkv, W_proj, trace=False)
    return out



# revision 56
# speedup vs baseline: 1.8120x; 1.0020x over previous
"""Trainium2 Bass kernel v2: multi-head attention (B=2, T=2048, C=2048, H=16, D=128).

Sharding: tensor-parallel over heads. 8 cores x 2 heads each.
  - W_qkv columns sliced per head-pair, W_proj rows sliced per head-pair.
  - Each core computes a partial output [B*T, C]; host sums the 8 partials.

v3 changes vs v2 (435us):
  - out DMA'd straight from PSUM (f32) on the gpsimd queue: kills 256
    out-casts (~170us DVE) and moves DIRECT2D dispatch off the Scalar
    queue so exp ACTs never queue behind store dispatch.
  - finalize decoupled: unnormalized y copied out of PSUM immediately
    (ACT), reciprocal chain runs off the critical path, normalize is a
    late in-place bf16 DVE mul.  Next qt's MM2s no longer wait on the
    reciprocal DMA roundtrip.
  - dacc quads folded 4->1 on DVE before the ones-matmul: denominator
    contraction is 512 PE rows instead of 2048 (-10us PE).
  - bf16 rope tables (halves table DMA), wp DMA deferred past P1.
  - progressive ck-parts for the first proj tile so the first matmul
    only waits on the first w/x chunk DMAs, not 8 of them.
"""

import math

import numpy as np

N_CORES = 8
B, T, C = 2, 2048, 2048
N_HEAD, D = 16, 128
HPC = N_HEAD // N_CORES          # heads per core
JC = HPC * D                     # per-core slice width of qkv/proj dims

RP = 256                         # proj token tile (moving free dim)
RT = 512                         # attention query tile
KB = 128                         # key block (contraction tile)

# rope table dtype: "bf16" saves SBUF + DMA, needs mixed-dtype TT support
TABLE_DT = "bf16"


def _build(Bp, Tp, Cp, hpc, d):
    import concourse.bacc as bacc
    import concourse.tile as tile
    from concourse import mybir

    f32 = mybir.dt.float32
    bf16 = mybir.dt.bfloat16
    Exp = mybir.ActivationFunctionType.Exp
    Ln = mybir.ActivationFunctionType.Ln
    Copy = mybir.ActivationFunctionType.Copy

    jc = hpc * d
    BT = Bp * Tp
    n_ck = Cp // 128             # contraction chunks for proj
    n_rt = Tp // RP              # proj token tiles per batch
    n_sub = RP // 128            # v sub-blocks per proj tile
    n_kb = Tp // KB              # key blocks per batch
    n_kbp = n_kb // 2            # key-block pairs
    n_qt = Tp // RT              # query tiles per batch
    n_rb = Tp // 128             # row blocks for out proj
    n_ot = Cp // RT              # output column tiles
    scale = 1.0 / math.sqrt(d)
    hd = d // 2
    tdt = bf16 if TABLE_DT == "bf16" else f32

    nc = bacc.Bacc("TRN2", target_bir_lowering=False, debug=False)

    xT = nc.declare_dram_parameter("xT", [Cp, BT], bf16, isOutput=False)
    wqkv = nc.declare_dram_parameter("wqkv", [Cp, 3 * jc], bf16,
                                     isOutput=False)
    wp = nc.declare_dram_parameter("wp", [jc, Cp], bf16, isOutput=False)
    ones_d = nc.declare_dram_parameter("ones", [128, 1], bf16, isOutput=False)
    cosT = nc.declare_dram_parameter("cosT", [d, Tp], tdt, isOutput=False)
    sinT = nc.declare_dram_parameter("sinT", [d, Tp], tdt, isOutput=False)
    out = nc.declare_dram_parameter("out", [BT, Cp], bf16, isOutput=True)

    with tile.TileContext(nc) as tc:
        with (
            nc.allow_low_precision(reason="bf16 staging, f32 PSUM accum"),
            tc.tile_pool(name="wpool", bufs=1) as wpool,
            tc.tile_pool(name="acts", bufs=1) as acts,
            tc.tile_pool(name="xpool", bufs=17) as xpool,
            tc.tile_pool(name="rope", bufs=4) as ropep,
            tc.tile_pool(name="epool", bufs=4) as epool,
            tc.tile_pool(name="dpool", bufs=3) as dpool,
            tc.tile_pool(name="small", bufs=2) as small,
            tc.tile_pool(name="opool", bufs=6) as opool,
            tc.tile_pool(name="pss", bufs=2, space="PSUM") as pss,
            tc.tile_pool(name="psy", bufs=1, space="PSUM") as psy,
        ):
            psp_cm = tc.tile_pool(name="psp", bufs=1, space="PSUM")
            psp = psp_cm.__enter__()
            psoA_cm = tc.tile_pool(name="psoA", bufs=1, space="PSUM")
            pso = psoA_cm.__enter__()
            # ---- resident weights / tables ----
            # inputs (w, x) stream on the sync/SP queue; tables + wp + all
            # output traffic go on the scalar queue so they never starve
            # the x-tile stream.
            wq_sb, wk_sb, wv_sb = [], [], []
            xpair = {}
            # DMA dispatch is ~600ns per dma_start per sequencer, and one
            # HW queue moves only ~25GB/s -- so the startup is shaped by
            # (a) dispatch counts per sequencer and (b) per-queue prefix
            # bytes.  x tiles go as ck-pair slabs on sync, w chunks are
            # dispatched from the (early-idle) vector queue, and the big
            # table loads are split into 128KB pieces on scalar.  (DMA
            # can only be initiated from sync/scalar/gpsimd; gpsimd is
            # idle during startup so it carries the w dispatches.)
            for j in range(n_ck // 2):
                xp = xpool.tile([128, 2, RP], bf16, tag="xs", bufs=6,
                                name=f"xtpre{j}")
                nc.sync.dma_start(
                    xp, xT[2 * j * 128:(2 * j + 2) * 128, 0:RP].rearrange(
                        "(g p) t -> p g t", p=128))
                xpair[("s", 0, j)] = xp
            for ck in range(n_ck):
                t = wpool.tile([128, 3 * jc], bf16, tag=f"w{ck}",
                               name=f"w{ck}")
                nc.gpsimd.dma_start(t, wqkv[ck * 128:(ck + 1) * 128, :])
                wq_sb.append(t[:, 0:jc])
                wk_sb.append(t[:, jc:2 * jc])
                wv_sb.append(t[:, 2 * jc:3 * jc])
            ones_sb = wpool.tile([128, 1], bf16, tag="ones")
            nc.sync.dma_start(ones_sb, ones_d[:])
            # doubled rope tables for h-batched rope: [d, hpc, Tp]
            cos2 = wpool.tile([d, hpc, Tp], tdt, tag="cos2")
            sin2 = wpool.tile([d, hpc, Tp], tdt, tag="sin2")
            qT = Tp // 4
            for h in range(hpc):
                for p in range(4):
                    psl = slice(p * qT, (p + 1) * qT)
                    nc.scalar.dma_start(cos2[:, h, psl], cosT[:, psl])
                    nc.scalar.dma_start(sin2[:, h, psl], sinT[:, psl])
            wp_sb = wpool.tile([128, hpc, Cp], bf16, tag="wp")

            # per-batch activation tensors (both batches resident -> the
            # scheduler can overlap attn(b) with proj(b+1))
            qT_sb = [acts.tile([128, hpc, Tp], bf16, tag=f"qT{b}",
                               name=f"qT{b}") for b in range(Bp)]
            kT_sb = [acts.tile([128, hpc, Tp], bf16, tag=f"kT{b}",
                               name=f"kT{b}") for b in range(Bp)]
            v_sb = [acts.tile([128, n_kb, jc], bf16, tag=f"v{b}",
                              name=f"v{b}") for b in range(Bp)]
            yT_sb = [acts.tile([128, hpc, Tp], bf16, tag=f"yT{b}",
                               name=f"yT{b}") for b in range(Bp)]

            def proj_tile_range(b, rt, ck_lo, ck_hi, state):
                """qkv projection for one RP-wide token tile + rope,
                emitted in ck-range parts so attention work can weave
                between them at fine grain."""
                tsl = slice(rt * RP, (rt + 1) * RP)
                if ck_lo == 0:
                    state["q"] = psp.tile([128, hpc, RP], f32, tag="qps",
                                          name=f"qps{b}_{rt}")
                    state["k"] = psp.tile([128, hpc, RP], f32, tag="kps",
                                          name=f"kps{b}_{rt}")
                    state["v"] = psp.tile([128, n_sub, jc], f32, tag="vps",
                                          name=f"vps{b}_{rt}")
                q_ps, k_ps, v_ps = state["q"], state["k"], state["v"]
                for ck in range(ck_lo, ck_hi):
                    if b == 0 and rt < 2:
                        # first two token tiles: per-rt ck-pair x tiles
                        # so each proj part waits on the minimum queue
                        # prefix during the DMA-limited startup
                        pk = ("s", rt, ck // 2)
                        if pk not in xpair:
                            xp = xpool.tile([128, 2, RP], bf16, tag="xs",
                                            bufs=6)
                            c0 = (ck // 2) * 2 * 128
                            p0 = rt * RP
                            nc.sync.dma_start(
                                xp, xT[c0:c0 + 256,
                                       p0:p0 + RP].rearrange(
                                           "(g p) t -> p g t", p=128))
                            xpair[pk] = xp
                        xt = xpair[pk][:, ck % 2, :]
                    else:
                        # x streamed in [128, 2*RP] pair-slabs
                        pk = (b, rt // 2, ck)
                        if pk not in xpair:
                            xp = xpool.tile([128, 2 * RP], bf16, tag="xt")
                            p0 = b * Tp + (rt // 2) * 2 * RP
                            nc.sync.dma_start(
                                xp, xT[ck * 128:(ck + 1) * 128,
                                       p0:p0 + 2 * RP])
                            xpair[pk] = xp
                        xt = xpair[pk][:, (rt % 2) * RP:(rt % 2) * RP + RP]
                    first = ck == 0
                    last = ck == n_ck - 1
                    for h in range(hpc):
                        nc.tensor.matmul(
                            q_ps[:, h, :],
                            wq_sb[ck][:, h * d:(h + 1) * d],
                            xt, start=(first and h == 0),
                            stop=(last and h == hpc - 1),
                            skip_group_check=True)
                        nc.tensor.matmul(
                            k_ps[:, h, :],
                            wk_sb[ck][:, h * d:(h + 1) * d],
                            xt, start=(first and h == 0),
                            stop=(last and h == hpc - 1),
                            skip_group_check=True)
                    for s in range(n_sub):
                        nc.tensor.matmul(
                            v_ps[:, s, :],
                            xt[:, s * 128:(s + 1) * 128],
                            wv_sb[ck], start=(first and s == 0),
                            stop=(last and s == n_sub - 1),
                            skip_group_check=True)
                if ck_hi != n_ck:
                    return
                # h-batched rope epilogue on [128, hpc*RP]:
                #   dst = psum*cos2 + swap(psum)*sin2_signed
                for ps, dst in ((q_ps, qT_sb[b]), (k_ps, kT_sb[b])):
                    c2 = cos2[:, :, tsl]
                    s2 = sin2[:, :, tsl]
                    t1 = ropep.tile([d, hpc, RP], f32, tag="t1")
                    nc.vector.tensor_mul(t1, ps, c2)
                    t2 = ropep.tile([d, hpc, RP], f32, tag="t2")
                    nc.vector.tensor_mul(t2[0:hd], ps[hd:d], s2[0:hd])
                    nc.vector.tensor_mul(t2[hd:d], ps[0:hd], s2[hd:d])
                    nc.vector.tensor_add(dst[:, :, tsl], t1, t2)
                # v copy (both sub-blocks in one ACT instruction)
                nc.scalar.activation(
                    v_sb[b][:, rt * n_sub:(rt + 1) * n_sub, :], v_ps, Copy)

            def proj_tile(b, rt):
                st = {}
                for part in range(4):
                    proj_tile_range(b, rt, part * 4, (part + 1) * 4, st)

            def attn_unit(b, qt, kb, h, y_ps, daccs, equads, pend):
                """one (query-tile, key-block, head) attention step.

                s tiles are single-bank [128, RT], double-buffered -> the
                next MM1 overlaps the current exp.  e tiles are quads
                [128, 4, RT] shared by 4 consecutive key blocks so dacc
                accumulates 2048 elems per DVE op.
                """
                qsl = slice(qt * RT, (qt + 1) * RT)
                s_ps = pss.tile([128, RT], f32, tag="s")
                nc.tensor.matmul(
                    s_ps,
                    kT_sb[b][:, h, kb * KB:(kb + 1) * KB],
                    qT_sb[b][:, h, qsl],
                    start=True, stop=True, skip_group_check=True)
                if kb % 4 == 0:
                    equads[h] = epool.tile([128, 4, RT], bf16, tag="e",
                                           name=f"e{b}_{qt}_{h}_{kb}")
                eq = equads[h]
                nc.scalar.activation(eq[:, kb % 4, :], s_ps, Exp,
                                     scale=scale)
                if kb % 4 == 3:
                    qd = kb // 4
                    if qd == 0:
                        nc.vector.tensor_copy(out=daccs[h], in_=eq)
                    else:
                        nc.vector.tensor_add(daccs[h], daccs[h], eq)
                pend[h].append((kb, eq))

            def mm2_drain(b, h, y_ps, pend, keep):
                while len(pend[h]) > keep:
                    kb, eq = pend[h].pop(0)
                    nc.tensor.matmul(
                        y_ps[h],
                        v_sb[b][:, kb, h * d:(h + 1) * d],
                        eq[:, kb % 4, :],
                        start=(kb == 0), stop=(kb == n_kb - 1),
                        skip_group_check=True)

            def finalize(b, qt, h, y_ps, daccs, fused=False):
                qsl = slice(qt * RT, (qt + 1) * RT)
                # free the PSUM bank immediately: unnormalized copy; the
                # reciprocal chain below runs off the critical path and
                # the normalize is a late in-place bf16 mul.  (fused path
                # for the very last tile: keep y in PSUM, one less link.)
                if not fused:
                    nc.scalar.activation(yT_sb[b][:, h, qsl], y_ps[h], Copy)
                # fold dacc quads 4->1 so the ones-matmul is 512 rows
                f2 = small.tile([128, 2, RT], bf16, tag="f2")
                nc.vector.tensor_add(f2, daccs[h][:, 0:2, :],
                                     daccs[h][:, 2:4, :])
                fold = small.tile([128, RT], bf16, tag="fold")
                nc.vector.tensor_add(fold, f2[:, 0, :], f2[:, 1, :])
                dsum = pso.tile([1, RT], f32, tag="fin", bufs=1,
                                name=f"dsum{b}_{qt}_{h}")
                nc.tensor.matmul(dsum, ones_sb, fold, start=True,
                                 stop=True, skip_group_check=True)
                # single-op DVE Newton reciprocal straight off PSUM (no
                # DMA-spread roundtrip); 18 correct bits, plenty for bf16
                rec = small.tile([1, RT], f32, tag="rec")
                nc.vector.reciprocal_approx_fast(out=rec, in_=dsum)
                if fused:
                    # tail path: broadcast f32 directly (skip the bf16
                    # recast hop) and multiply straight out of PSUM
                    bcf = small.tile([128, RT], f32, tag="bcf", bufs=1)
                    nc.gpsimd.partition_broadcast(out_ap=bcf, in_ap=rec)
                    nc.vector.tensor_mul(yT_sb[b][:, h, qsl], y_ps[h],
                                         bcf)
                else:
                    rec_bf = small.tile([1, RT], bf16, tag="recb")
                    nc.scalar.activation(rec_bf, rec, Copy)
                    bc = small.tile([128, RT], bf16, tag="bc")
                    nc.gpsimd.partition_broadcast(out_ap=bc, in_ap=rec_bf)
                    nc.vector.tensor_mul(yT_sb[b][:, h, qsl],
                                         yT_sb[b][:, h, qsl], bc)

            def attn_qt(b, qt, interleave=None, defer_last_fin=False,
                        fused_tail=False):
                """all attention work for one query tile; interleave is a
                list of thunks emitted between key-block units.

                h-outer: head 0 finishes halfway through the tile so its
                finalize chain (reciprocal etc.) hides under head 1's
                attention units instead of stalling the next tile.  The
                last head's finalize can be deferred (returned as a thunk
                for the caller to weave into the NEXT tile) so its dsum
                matmul never head-of-line blocks the in-order PE queue
                while the DVE fold catches up."""
                y_ps = [psy.tile([d, RT], f32, tag=f"y{h}",
                                 name=f"y{b}_{qt}_{h}")
                        for h in range(hpc)]
                daccs = [dpool.tile([128, 4, RT], bf16, tag="dacc",
                                    name=f"dacc{b}_{qt}_{h}")
                         for h in range(hpc)]
                equads = [None] * hpc
                pend = [[] for _ in range(hpc)]
                il = list(interleave or [])
                # spread the filler thunks across the unit sequence
                nu = hpc * n_kb
                points = {}
                for i, th in enumerate(il):
                    points.setdefault(min(nu - 1, i * nu // len(il)),
                                      []).append(th)
                u = 0
                for h in range(hpc):
                    for kb in range(n_kb):
                        attn_unit(b, qt, kb, h, y_ps, daccs, equads, pend)
                        # lag the MM2s two key-blocks behind their exp so
                        # the PE never head-of-line blocks on ScalarE
                        mm2_drain(b, h, y_ps, pend, 2)
                        for th in points.get(u, []):
                            th()
                        u += 1
                    mm2_drain(b, h, y_ps, pend, 0)
                    if h < hpc - 1:
                        finalize(b, qt, h, y_ps, daccs)
                hl = hpc - 1
                fin = (lambda: finalize(b, qt, hl, y_ps, daccs,
                                        fused=fused_tail))
                if defer_last_fin:
                    return fin
                fin()
                return None

            def outproj_unit(b, rb, ot, eng):
                o_ps = pso.tile([128, RT], f32, tag="o")
                for h in range(hpc):
                    nc.tensor.matmul(
                        o_ps,
                        yT_sb[b][:, h, rb * 128:(rb + 1) * 128],
                        wp_sb[:, h, ot * RT:(ot + 1) * RT],
                        start=(h == 0), stop=(h == hpc - 1),
                        skip_group_check=True)
                # cast on DVE/ACT (gpsimd has no PSUM port); the DIRECT2D
                # dispatch rides the sync queue -- idle in P3 (the x
                # stream is done) -- so exp ACTs and broadcasts never
                # queue behind store dispatch
                o_sb = opool.tile([128, RT], bf16, tag="o")
                if eng == 0:
                    nc.vector.tensor_copy(out=o_sb, in_=o_ps)
                else:
                    nc.scalar.activation(o_sb, o_ps, Copy)
                nc.sync.dma_start(
                    out[b * Tp + rb * 128:b * Tp + (rb + 1) * 128,
                        ot * RT:(ot + 1) * RT],
                    o_sb)

            # ================= emission schedule =================
            ppb = n_rt // n_qt          # proj tiles per query tile
            opb = n_rb // n_qt          # row blocks per query tile
            cnt = [0]
            # P1: proj b0 with attn(b0, qt0) streaming kb-wise behind
            # the tiles that produce its k/v blocks (fills the otherwise
            # idle ScalarE and covers DMA stalls with PE work)
            yq0 = [psy.tile([d, RT], f32, tag=f"y{h}", name=f"yq0_{h}")
                   for h in range(hpc)]
            dq0 = [dpool.tile([128, 4, RT], bf16, tag="dacc",
                              name=f"daccq0_{h}") for h in range(hpc)]
            eq0 = [None] * hpc
            pq0 = [[] for _ in range(hpc)]
            for rt in range(n_rt):
                if rt == 0:
                    # progressive ck-parts: the first matmuls wait only on
                    # the first one or two w/x chunk DMAs
                    st0 = {}
                    for lo, hi in ((0, 1), (1, 2), (2, 4), (4, 8), (8, 16)):
                        proj_tile_range(0, 0, lo, hi, st0)
                else:
                    proj_tile(0, rt)
                if rt == 1:
                    # wp is first used in P3; deferring its 1MB DMA keeps
                    # early HBM bandwidth for the w/x critical path, and
                    # quarter-pieces keep HW-queue prefixes short
                    wp_r = wp.rearrange("(h p) o -> p h o", p=128)
                    for p in range(4):
                        psl = slice(p * 512, (p + 1) * 512)
                        nc.scalar.dma_start(wp_sb[:, :, psl],
                                            wp_r[:, :, psl])
                if rt >= 2:
                    for kb in (2 * (rt - 2), 2 * (rt - 2) + 1):
                        for h in range(hpc):
                            attn_unit(0, 0, kb, h, yq0, dq0, eq0, pq0)
                            mm2_drain(0, h, yq0, pq0, 2)
            # finish qt0's remaining key blocks, then finalize it
            for kb in range(2 * (n_rt - 2), n_kb):
                for h in range(hpc):
                    attn_unit(0, 0, kb, h, yq0, dq0, eq0, pq0)
                    mm2_drain(0, h, yq0, pq0, 2)
            for h in range(hpc):
                mm2_drain(0, h, yq0, pq0, 0)
            for h in range(hpc - 1):
                finalize(0, 0, h, yq0, dq0)
            pend_fin = lambda: finalize(0, 0, hpc - 1, yq0, dq0)
            # P2: attn b0 qt1-3 interleaved with ALL 8 proj-b1 tiles
            # (quarter-tile weave, distributed across the 3 query tiles)
            # plus the previous tile's b0 outproj rows: P2 is PE-bound
            # with DVE slack, so the store casts are free here, and P3
            # (where DVE is co-critical) keeps only the b1 stores.
            # The previous tile's deferred finalize leads each weave.
            def op_thunks(units, alt=False):
                ths = []
                for i in range(0, len(units), 2):
                    chunk = units[i:i + 2]
                    def th(chunk=chunk):
                        for j, (b_, rb, ot) in enumerate(chunk):
                            cnt[0] += 1
                            outproj_unit(b_, rb, ot,
                                         (ot % 2) if alt else 0)
                    ths.append(th)
                return ths
            noop = lambda: None
            def mix(a, bl):
                out, ia, ib = [], 0, 0
                while ia < len(a) or ib < len(bl):
                    if ia * (len(bl) + 1) <= ib * (len(a) + 1):
                        if ia < len(a):
                            out.append(a[ia])
                        ia += 1
                    else:
                        if ib < len(bl):
                            out.append(bl[ib])
                        ib += 1
                return out
            for qt in range(1, n_qt):
                j = qt - 1
                pthunks = []
                for rt in range(j * n_rt // 3, (j + 1) * n_rt // 3):
                    st = {}
                    for part in range(4):
                        pthunks.append(
                            lambda rt=rt, part=part, st=st:
                                proj_tile_range(1, rt, part * 4,
                                                (part + 1) * 4, st))
                thunks = [pend_fin] + pthunks
                pend_fin = attn_qt(0, qt, interleave=thunks,
                                   defer_last_fin=True)
            # proj + qkv PSUM done -> release banks for the P3 pool
            psoA_cm.__exit__(None, None, None)
            psp_cm.__exit__(None, None, None)
            psoB_cm = tc.tile_pool(name="psoB", bufs=3, space="PSUM")
            pso = psoB_cm.__enter__()
            # P3: attn b1; outproj woven between units: b0 rows early
            # (long finalized), the previous tile's b1 rows late
            for qt in range(n_qt):
                units = [(0, rb, ot)
                         for rb in range(qt * opb, (qt + 1) * opb)
                         for ot in range(n_ot)]
                units1 = [(1, rb, ot)
                          for rb in range((qt - 1) * opb, qt * opb)
                          for ot in range(n_ot)] if qt > 0 else []
                last = qt == n_qt - 1
                # on the last tile the late stores' casts alternate onto
                # ACT (its exps are done by then) so the DVE queue can
                # reach the tail finalize's folds sooner
                thunks = ([pend_fin, noop] + op_thunks(units)
                          + op_thunks(units1, alt=last))
                pend_fin = attn_qt(1, qt, interleave=thunks,
                                   defer_last_fin=not last,
                                   fused_tail=last)
            # P4: last query tile's outproj b1; alternate cast engines
            # (ACT is idle here) so the drain is not CAST-bound
            for rb in range((n_qt - 1) * opb, n_qt * opb):
                for ot in range(n_ot):
                    cnt[0] += 1
                    outproj_unit(1, rb, ot, ot % 2)
            psoB_cm.__exit__(None, None, None)

    nc.compile()
    return nc


def _prep_in_maps(x, cos, sin, W_qkv, W_proj, n_cores, hpc, d):
    """Host-side shard prep: pure layout work (transpose / slice / sign fold)."""
    Bp, Tp, Cp = x.shape
    jc = hpc * d
    import ml_dtypes
    tdt = ml_dtypes.bfloat16 if TABLE_DT == "bf16" else np.float32
    xTa = np.ascontiguousarray(x.reshape(Bp * Tp, Cp).T).astype(ml_dtypes.bfloat16)
    cosT = np.ascontiguousarray(cos.T).astype(tdt)
    sinT = np.ascontiguousarray(sin.T).copy()
    sinT[: d // 2] *= -1.0
    sinT = sinT.astype(tdt)
    in_maps = []
    for c in range(n_cores):
        j0, j1 = c * jc, (c + 1) * jc
        in_maps.append({
            "xT": xTa,
            "wqkv": np.ascontiguousarray(np.concatenate(
                [W_qkv[:, j0:j1], W_qkv[:, Cp + j0:Cp + j1],
                 W_qkv[:, 2 * Cp + j0:2 * Cp + j1]], axis=1,
            )).astype(ml_dtypes.bfloat16),
            "wp": np.ascontiguousarray(W_proj[j0:j1, :]).astype(ml_dtypes.bfloat16),
            "ones": np.ones((128, 1), dtype=ml_dtypes.bfloat16),
            "cosT": cosT,
            "sinT": sinT,
        })
    return in_maps


def _install_ntff_hook():
    """Enable NTFF profiling under axon when the boot image lacks the
    antenv.axon_hooks shim. Harmless if anything is missing."""
    import sys
    import types
    try:
        from antenv.axon_hooks import get_axon_ntff_profile_hook
        if get_axon_ntff_profile_hook() is not None:
            return
    except ImportError:
        pass
    try:
        sys.path.insert(0, "/root/.axon_site")
        from trn_agent_boot.trn_boot import _ntff_profile_via_ctypes

        hook = _ntff_profile_via_ctypes("/opt/axon/libaxon_pjrt.so")
        if hook is None:
            return
        mod = types.ModuleType("antenv.axon_hooks")
        mod.get_axon_ntff_profile_hook = lambda: hook
        mod.set_axon_ntff_profile_hook = lambda h: None
        import antenv
        antenv.axon_hooks = mod
        sys.modules["antenv.axon_hooks"] = mod
    except Exception:
        pass


def _run(x, cos, sin, W_qkv, W_proj, trace=False):
    from concourse.bass_utils import run_bass_kernel_spmd

    if trace:
        _install_ntff_hook()

    x = np.ascontiguousarray(x, dtype=np.float32)
    cos = np.ascontiguousarray(cos, dtype=np.float32)
    sin = np.ascontiguousarray(sin, dtype=np.float32)
    W_qkv = np.ascontiguousarray(W_qkv, dtype=np.float32)
    W_proj = np.ascontiguousarray(W_proj, dtype=np.float32)

    Bp, Tp, Cp = x.shape
    nc = _build(Bp, Tp, Cp, HPC, D)
    in_maps = _prep_in_maps(x, cos, sin, W_qkv, W_proj, N_CORES, HPC, D)
    res = run_bass_kernel_spmd(nc, in_maps, core_ids=list(range(N_CORES)),
                               trace=trace)
    acc = np.zeros((Bp * Tp, Cp), dtype=np.float32)
    for i in range(N_CORES):
        acc += np.asarray(res.results[i]["out"], dtype=np.float32)
    return acc.reshape(Bp, Tp, Cp), res


def kernel(x, cos, sin, W_qkv, W_proj):
    out, _ = _run(x, cos, sin, W_qkv, W_proj, trace=False)
    return out



# revision 58
# speedup vs baseline: 1.8144x; 1.0014x over previous
"""Trainium2 Bass kernel v2: multi-head attention (B=2, T=2048, C=2048, H=16, D=128).

Sharding: tensor-parallel over heads. 8 cores x 2 heads each.
  - W_qkv columns sliced per head-pair, W_proj rows sliced per head-pair.
  - Each core computes a partial output [B*T, C]; host sums the 8 partials.

v3 changes vs v2 (435us):
  - out DMA'd straight from PSUM (f32) on the gpsimd queue: kills 256
    out-casts (~170us DVE) and moves DIRECT2D dispatch off the Scalar
    queue so exp ACTs never queue behind store dispatch.
  - finalize decoupled: unnormalized y copied out of PSUM immediately
    (ACT), reciprocal chain runs off the critical path, normalize is a
    late in-place bf16 DVE mul.  Next qt's MM2s no longer wait on the
    reciprocal DMA roundtrip.
  - dacc quads folded 4->1 on DVE before the ones-matmul: denominator
    contraction is 512 PE rows instead of 2048 (-10us PE).
  - bf16 rope tables (halves table DMA), wp DMA deferred past P1.
  - progressive ck-parts for the first proj tile so the first matmul
    only waits on the first w/x chunk DMAs, not 8 of them.
"""

import math

import numpy as np

N_CORES = 8
B, T, C = 2, 2048, 2048
N_HEAD, D = 16, 128
HPC = N_HEAD // N_CORES          # heads per core
JC = HPC * D                     # per-core slice width of qkv/proj dims

RP = 256                         # proj token tile (moving free dim)
RT = 512                         # attention query tile
KB = 128                         # key block (contraction tile)

# rope table dtype: "bf16" saves SBUF + DMA, needs mixed-dtype TT support
TABLE_DT = "bf16"


def _build(Bp, Tp, Cp, hpc, d):
    import concourse.bacc as bacc
    import concourse.tile as tile
    from concourse import mybir

    f32 = mybir.dt.float32
    bf16 = mybir.dt.bfloat16
    Exp = mybir.ActivationFunctionType.Exp
    Ln = mybir.ActivationFunctionType.Ln
    Copy = mybir.ActivationFunctionType.Copy

    jc = hpc * d
    BT = Bp * Tp
    n_ck = Cp // 128             # contraction chunks for proj
    n_rt = Tp // RP              # proj token tiles per batch
    n_sub = RP // 128            # v sub-blocks per proj tile
    n_kb = Tp // KB              # key blocks per batch
    n_kbp = n_kb // 2            # key-block pairs
    n_qt = Tp // RT              # query tiles per batch
    n_rb = Tp // 128             # row blocks for out proj
    n_ot = Cp // RT              # output column tiles
    scale = 1.0 / math.sqrt(d)
    hd = d // 2
    tdt = bf16 if TABLE_DT == "bf16" else f32

    nc = bacc.Bacc("TRN2", target_bir_lowering=False, debug=False)

    xT = nc.declare_dram_parameter("xT", [Cp, BT], bf16, isOutput=False)
    wqkv = nc.declare_dram_parameter("wqkv", [Cp, 3 * jc], bf16,
                                     isOutput=False)
    wp = nc.declare_dram_parameter("wp", [jc, Cp], bf16, isOutput=False)
    ones_d = nc.declare_dram_parameter("ones", [128, 1], bf16, isOutput=False)
    cosT = nc.declare_dram_parameter("cosT", [d, Tp], tdt, isOutput=False)
    sinT = nc.declare_dram_parameter("sinT", [d, Tp], tdt, isOutput=False)
    out = nc.declare_dram_parameter("out", [BT, Cp], bf16, isOutput=True)

    with tile.TileContext(nc) as tc:
        with (
            nc.allow_low_precision(reason="bf16 staging, f32 PSUM accum"),
            tc.tile_pool(name="wpool", bufs=1) as wpool,
            tc.tile_pool(name="acts", bufs=1) as acts,
            tc.tile_pool(name="xpool", bufs=17) as xpool,
            tc.tile_pool(name="rope", bufs=4) as ropep,
            tc.tile_pool(name="epool", bufs=4) as epool,
            tc.tile_pool(name="dpool", bufs=3) as dpool,
            tc.tile_pool(name="small", bufs=2) as small,
            tc.tile_pool(name="opool", bufs=6) as opool,
            tc.tile_pool(name="pss", bufs=2, space="PSUM") as pss,
            tc.tile_pool(name="psy", bufs=1, space="PSUM") as psy,
        ):
            psp_cm = tc.tile_pool(name="psp", bufs=1, space="PSUM")
            psp = psp_cm.__enter__()
            psoA_cm = tc.tile_pool(name="psoA", bufs=1, space="PSUM")
            pso = psoA_cm.__enter__()
            # ---- resident weights / tables ----
            # inputs (w, x) stream on the sync/SP queue; tables + wp + all
            # output traffic go on the scalar queue so they never starve
            # the x-tile stream.
            wq_sb, wk_sb, wv_sb = [], [], []
            xpair = {}
            # DMA dispatch is ~600ns per dma_start per sequencer, and one
            # HW queue moves only ~25GB/s -- so the startup is shaped by
            # (a) dispatch counts per sequencer and (b) per-queue prefix
            # bytes.  x tiles go as ck-pair slabs on sync, w chunks are
            # dispatched from the (early-idle) vector queue, and the big
            # table loads are split into 128KB pieces on scalar.  (DMA
            # can only be initiated from sync/scalar/gpsimd; gpsimd is
            # idle during startup so it carries the w dispatches.)
            for j in range(n_ck // 2):
                xp = xpool.tile([128, 2, RP], bf16, tag="xs", bufs=6,
                                name=f"xtpre{j}")
                nc.sync.dma_start(
                    xp, xT[2 * j * 128:(2 * j + 2) * 128, 0:RP].rearrange(
                        "(g p) t -> p g t", p=128))
                xpair[("s", 0, j)] = xp
            for ck in range(n_ck):
                t = wpool.tile([128, 3 * jc], bf16, tag=f"w{ck}",
                               name=f"w{ck}")
                nc.gpsimd.dma_start(t, wqkv[ck * 128:(ck + 1) * 128, :])
                wq_sb.append(t[:, 0:jc])
                wk_sb.append(t[:, jc:2 * jc])
                wv_sb.append(t[:, 2 * jc:3 * jc])
            ones_sb = wpool.tile([128, 1], bf16, tag="ones")
            nc.sync.dma_start(ones_sb, ones_d[:])
            # doubled rope tables for h-batched rope: [d, hpc, Tp]
            cos2 = wpool.tile([d, hpc, Tp], tdt, tag="cos2")
            sin2 = wpool.tile([d, hpc, Tp], tdt, tag="sin2")
            qT = Tp // 4
            for h in range(hpc):
                for p in range(4):
                    psl = slice(p * qT, (p + 1) * qT)
                    nc.scalar.dma_start(cos2[:, h, psl], cosT[:, psl])
                    nc.scalar.dma_start(sin2[:, h, psl], sinT[:, psl])
            wp_sb = wpool.tile([128, hpc, Cp], bf16, tag="wp")

            # per-batch activation tensors (both batches resident -> the
            # scheduler can overlap attn(b) with proj(b+1))
            qT_sb = [acts.tile([128, hpc, Tp], bf16, tag=f"qT{b}",
                               name=f"qT{b}") for b in range(Bp)]
            kT_sb = [acts.tile([128, hpc, Tp], bf16, tag=f"kT{b}",
                               name=f"kT{b}") for b in range(Bp)]
            v_sb = [acts.tile([128, n_kb, jc], bf16, tag=f"v{b}",
                              name=f"v{b}") for b in range(Bp)]
            yT_sb = [acts.tile([128, hpc, Tp], bf16, tag=f"yT{b}",
                               name=f"yT{b}") for b in range(Bp)]

            def proj_tile_range(b, rt, ck_lo, ck_hi, state):
                """qkv projection for one RP-wide token tile + rope,
                emitted in ck-range parts so attention work can weave
                between them at fine grain."""
                tsl = slice(rt * RP, (rt + 1) * RP)
                if ck_lo == 0:
                    state["q"] = psp.tile([128, hpc, RP], f32, tag="qps",
                                          name=f"qps{b}_{rt}")
                    state["k"] = psp.tile([128, hpc, RP], f32, tag="kps",
                                          name=f"kps{b}_{rt}")
                    state["v"] = psp.tile([128, n_sub, jc], f32, tag="vps",
                                          name=f"vps{b}_{rt}")
                q_ps, k_ps, v_ps = state["q"], state["k"], state["v"]
                for ck in range(ck_lo, ck_hi):
                    if b == 0 and rt < 2:
                        # first two token tiles: per-rt ck-pair x tiles
                        # so each proj part waits on the minimum queue
                        # prefix during the DMA-limited startup
                        pk = ("s", rt, ck // 2)
                        if pk not in xpair:
                            xp = xpool.tile([128, 2, RP], bf16, tag="xs",
                                            bufs=6)
                            c0 = (ck // 2) * 2 * 128
                            p0 = rt * RP
                            nc.sync.dma_start(
                                xp, xT[c0:c0 + 256,
                                       p0:p0 + RP].rearrange(
                                           "(g p) t -> p g t", p=128))
                            xpair[pk] = xp
                        xt = xpair[pk][:, ck % 2, :]
                    else:
                        # x streamed in [128, 2*RP] pair-slabs
                        pk = (b, rt // 2, ck)
                        if pk not in xpair:
                            xp = xpool.tile([128, 2 * RP], bf16, tag="xt")
                            p0 = b * Tp + (rt // 2) * 2 * RP
                            nc.sync.dma_start(
                                xp, xT[ck * 128:(ck + 1) * 128,
                                       p0:p0 + 2 * RP])
                            xpair[pk] = xp
                        xt = xpair[pk][:, (rt % 2) * RP:(rt % 2) * RP + RP]
                    first = ck == 0
                    last = ck == n_ck - 1
                    for h in range(hpc):
                        nc.tensor.matmul(
                            q_ps[:, h, :],
                            wq_sb[ck][:, h * d:(h + 1) * d],
                            xt, start=(first and h == 0),
                            stop=(last and h == hpc - 1),
                            skip_group_check=True)
                        nc.tensor.matmul(
                            k_ps[:, h, :],
                            wk_sb[ck][:, h * d:(h + 1) * d],
                            xt, start=(first and h == 0),
                            stop=(last and h == hpc - 1),
                            skip_group_check=True)
                    for s in range(n_sub):
                        nc.tensor.matmul(
                            v_ps[:, s, :],
                            xt[:, s * 128:(s + 1) * 128],
                            wv_sb[ck], start=(first and s == 0),
                            stop=(last and s == n_sub - 1),
                            skip_group_check=True)
                if ck_hi != n_ck:
                    return
                # h-batched rope epilogue on [128, hpc*RP]:
                #   dst = psum*cos2 + swap(psum)*sin2_signed
                for ps, dst in ((q_ps, qT_sb[b]), (k_ps, kT_sb[b])):
                    c2 = cos2[:, :, tsl]
                    s2 = sin2[:, :, tsl]
                    t1 = ropep.tile([d, hpc, RP], f32, tag="t1")
                    nc.vector.tensor_mul(t1, ps, c2)
                    t2 = ropep.tile([d, hpc, RP], f32, tag="t2")
                    nc.vector.tensor_mul(t2[0:hd], ps[hd:d], s2[0:hd])
                    nc.vector.tensor_mul(t2[hd:d], ps[0:hd], s2[hd:d])
                    nc.vector.tensor_add(dst[:, :, tsl], t1, t2)
                # v copy (both sub-blocks in one ACT instruction)
                nc.scalar.activation(
                    v_sb[b][:, rt * n_sub:(rt + 1) * n_sub, :], v_ps, Copy)

            def proj_tile(b, rt):
                st = {}
                for part in range(4):
                    proj_tile_range(b, rt, part * 4, (part + 1) * 4, st)

            def attn_unit(b, qt, kb, h, y_ps, daccs, equads, pend):
                """one (query-tile, key-block, head) attention step.

                s tiles are single-bank [128, RT], double-buffered -> the
                next MM1 overlaps the current exp.  e tiles are quads
                [128, 4, RT] shared by 4 consecutive key blocks so dacc
                accumulates 2048 elems per DVE op.
                """
                qsl = slice(qt * RT, (qt + 1) * RT)
                s_ps = pss.tile([128, RT], f32, tag="s")
                nc.tensor.matmul(
                    s_ps,
                    kT_sb[b][:, h, kb * KB:(kb + 1) * KB],
                    qT_sb[b][:, h, qsl],
                    start=True, stop=True, skip_group_check=True)
                if kb % 4 == 0:
                    equads[h] = epool.tile([128, 4, RT], bf16, tag="e",
                                           name=f"e{b}_{qt}_{h}_{kb}")
                eq = equads[h]
                nc.scalar.activation(eq[:, kb % 4, :], s_ps, Exp,
                                     scale=scale)
                if kb % 4 == 3:
                    qd = kb // 4
                    if qd == 0:
                        nc.vector.tensor_copy(out=daccs[h], in_=eq)
                    else:
                        nc.vector.tensor_add(daccs[h], daccs[h], eq)
                pend[h].append((kb, eq))

            def mm2_drain(b, h, y_ps, pend, keep):
                while len(pend[h]) > keep:
                    kb, eq = pend[h].pop(0)
                    nc.tensor.matmul(
                        y_ps[h],
                        v_sb[b][:, kb, h * d:(h + 1) * d],
                        eq[:, kb % 4, :],
                        start=(kb == 0), stop=(kb == n_kb - 1),
                        skip_group_check=True)

            def finalize(b, qt, h, y_ps, daccs, fused=False):
                qsl = slice(qt * RT, (qt + 1) * RT)
                # free the PSUM bank immediately: unnormalized copy; the
                # reciprocal chain below runs off the critical path and
                # the normalize is a late in-place bf16 mul.  (fused path
                # for the very last tile: keep y in PSUM, one less link.)
                if not fused:
                    nc.scalar.activation(yT_sb[b][:, h, qsl], y_ps[h], Copy)
                # fold dacc quads 4->1 so the ones-matmul is 512 rows
                f2 = small.tile([128, 2, RT], bf16, tag="f2")
                nc.vector.tensor_add(f2, daccs[h][:, 0:2, :],
                                     daccs[h][:, 2:4, :])
                fold = small.tile([128, RT], bf16, tag="fold")
                nc.vector.tensor_add(fold, f2[:, 0, :], f2[:, 1, :])
                dsum = pso.tile([1, RT], f32, tag="fin", bufs=1,
                                name=f"dsum{b}_{qt}_{h}")
                nc.tensor.matmul(dsum, ones_sb, fold, start=True,
                                 stop=True, skip_group_check=True)
                # single-op DVE Newton reciprocal straight off PSUM (no
                # DMA-spread roundtrip); 18 correct bits, plenty for bf16
                rec = small.tile([1, RT], f32, tag="rec")
                nc.vector.reciprocal_approx_fast(out=rec, in_=dsum)
                if fused:
                    # tail path: broadcast f32 directly (skip the bf16
                    # recast hop) and multiply straight out of PSUM
                    bcf = small.tile([128, RT], f32, tag="bcf", bufs=1)
                    nc.gpsimd.partition_broadcast(out_ap=bcf, in_ap=rec)
                    nc.vector.tensor_mul(yT_sb[b][:, h, qsl], y_ps[h],
                                         bcf)
                else:
                    rec_bf = small.tile([1, RT], bf16, tag="recb")
                    nc.scalar.activation(rec_bf, rec, Copy)
                    bc = small.tile([128, RT], bf16, tag="bc")
                    nc.gpsimd.partition_broadcast(out_ap=bc, in_ap=rec_bf)
                    nc.vector.tensor_mul(yT_sb[b][:, h, qsl],
                                         yT_sb[b][:, h, qsl], bc)

            def attn_qt(b, qt, interleave=None, defer_last_fin=False,
                        fused_tail=False):
                """all attention work for one query tile; interleave is a
                list of thunks emitted between key-block units.

                h-outer: head 0 finishes halfway through the tile so its
                finalize chain (reciprocal etc.) hides under head 1's
                attention units instead of stalling the next tile.  The
                last head's finalize can be deferred (returned as a thunk
                for the caller to weave into the NEXT tile) so its dsum
                matmul never head-of-line blocks the in-order PE queue
                while the DVE fold catches up."""
                y_ps = [psy.tile([d, RT], f32, tag=f"y{h}",
                                 name=f"y{b}_{qt}_{h}")
                        for h in range(hpc)]
                daccs = [dpool.tile([128, 4, RT], bf16, tag="dacc",
                                    name=f"dacc{b}_{qt}_{h}")
                         for h in range(hpc)]
                equads = [None] * hpc
                pend = [[] for _ in range(hpc)]
                il = list(interleave or [])
                # spread the filler thunks across the unit sequence
                nu = hpc * n_kb
                points = {}
                for i, th in enumerate(il):
                    points.setdefault(min(nu - 1, i * nu // len(il)),
                                      []).append(th)
                u = 0
                for h in range(hpc):
                    for kb in range(n_kb):
                        attn_unit(b, qt, kb, h, y_ps, daccs, equads, pend)
                        # lag the MM2s two key-blocks behind their exp so
                        # the PE never head-of-line blocks on ScalarE
                        mm2_drain(b, h, y_ps, pend, 2)
                        for th in points.get(u, []):
                            th()
                        u += 1
                    mm2_drain(b, h, y_ps, pend, 0)
                    if h < hpc - 1:
                        finalize(b, qt, h, y_ps, daccs)
                hl = hpc - 1
                fin = (lambda: finalize(b, qt, hl, y_ps, daccs,
                                        fused=fused_tail))
                if defer_last_fin:
                    return fin
                fin()
                return None

            def outproj_unit(b, rb, ot, eng):
                o_ps = pso.tile([128, RT], f32, tag="o")
                for h in range(hpc):
                    nc.tensor.matmul(
                        o_ps,
                        yT_sb[b][:, h, rb * 128:(rb + 1) * 128],
                        wp_sb[:, h, ot * RT:(ot + 1) * RT],
                        start=(h == 0), stop=(h == hpc - 1),
                        skip_group_check=True)
                # cast on DVE/ACT (gpsimd has no PSUM port); the DIRECT2D
                # dispatch rides the sync queue -- idle in P3 (the x
                # stream is done) -- so exp ACTs and broadcasts never
                # queue behind store dispatch
                o_sb = opool.tile([128, RT], bf16, tag="o")
                if eng == 0:
                    nc.vector.tensor_copy(out=o_sb, in_=o_ps)
                else:
                    nc.scalar.activation(o_sb, o_ps, Copy)
                nc.sync.dma_start(
                    out[b * Tp + rb * 128:b * Tp + (rb + 1) * 128,
                        ot * RT:(ot + 1) * RT],
                    o_sb)

            # ================= emission schedule =================
            ppb = n_rt // n_qt          # proj tiles per query tile
            opb = n_rb // n_qt          # row blocks per query tile
            cnt = [0]
            # P1: proj b0 with attn(b0, qt0) streaming kb-wise behind
            # the tiles that produce its k/v blocks (fills the otherwise
            # idle ScalarE and covers DMA stalls with PE work)
            yq0 = [psy.tile([d, RT], f32, tag=f"y{h}", name=f"yq0_{h}")
                   for h in range(hpc)]
            dq0 = [dpool.tile([128, 4, RT], bf16, tag="dacc",
                              name=f"daccq0_{h}") for h in range(hpc)]
            eq0 = [None] * hpc
            pq0 = [[] for _ in range(hpc)]
            # catch-up weave: after each proj tile, run every qt0
            # attention unit whose k/v blocks exist -- attention reads
            # only SBUF, so it soaks up the PE stalls while the DMA
            # engines are saturated with the w/x startup stream
            next_kb = [0]
            def qt0_units(upto_kb):
                while next_kb[0] <= min(upto_kb, n_kb - 1):
                    kb = next_kb[0]
                    for h in range(hpc):
                        attn_unit(0, 0, kb, h, yq0, dq0, eq0, pq0)
                        mm2_drain(0, h, yq0, pq0, 2)
                    next_kb[0] += 1
            for rt in range(n_rt):
                if rt == 0:
                    # progressive ck-parts: the first matmuls wait only on
                    # the first one or two w/x chunk DMAs
                    st0 = {}
                    for lo, hi in ((0, 1), (1, 2), (2, 4), (4, 8), (8, 16)):
                        proj_tile_range(0, 0, lo, hi, st0)
                else:
                    st0 = {}
                    for part in range(4):
                        proj_tile_range(0, rt, part * 4, (part + 1) * 4,
                                        st0)
                        if rt >= 2 and part == 1:
                            qt0_units(2 * rt - 1)
                if rt >= 1:
                    qt0_units(2 * rt + 1)
            qt0_units(n_kb - 1)
            for h in range(hpc):
                mm2_drain(0, h, yq0, pq0, 0)
            for h in range(hpc - 1):
                finalize(0, 0, h, yq0, dq0)
            pend_fin = lambda: finalize(0, 0, hpc - 1, yq0, dq0)
            # P2: attn b0 qt1-3 interleaved with ALL 8 proj-b1 tiles
            # (quarter-tile weave, distributed across the 3 query tiles)
            # plus the previous tile's b0 outproj rows: P2 is PE-bound
            # with DVE slack, so the store casts are free here, and P3
            # (where DVE is co-critical) keeps only the b1 stores.
            # The previous tile's deferred finalize leads each weave.
            def op_thunks(units, alt=False):
                ths = []
                for i in range(0, len(units), 2):
                    chunk = units[i:i + 2]
                    def th(chunk=chunk):
                        for j, (b_, rb, ot) in enumerate(chunk):
                            cnt[0] += 1
                            outproj_unit(b_, rb, ot,
                                         (ot % 2) if alt else 0)
                    ths.append(th)
                return ths
            noop = lambda: None
            def mix(a, bl):
                out, ia, ib = [], 0, 0
                while ia < len(a) or ib < len(bl):
                    if ia * (len(bl) + 1) <= ib * (len(a) + 1):
                        if ia < len(a):
                            out.append(a[ia])
                        ia += 1
                    else:
                        if ib < len(bl):
                            out.append(bl[ib])
                        ib += 1
                return out
            for qt in range(1, n_qt):
                if qt == 2:
                    # wp is first used in P3; load it here, well past the
                    # DMA-saturated startup window
                    wp_r = wp.rearrange("(h p) o -> p h o", p=128)
                    for p in range(4):
                        psl = slice(p * 512, (p + 1) * 512)
                        nc.scalar.dma_start(wp_sb[:, :, psl],
                                            wp_r[:, :, psl])
                j = qt - 1
                pthunks = []
                for rt in range(j * n_rt // 3, (j + 1) * n_rt // 3):
                    st = {}
                    for part in range(4):
                        pthunks.append(
                            lambda rt=rt, part=part, st=st:
                                proj_tile_range(1, rt, part * 4,
                                                (part + 1) * 4, st))
                thunks = [pend_fin] + pthunks
                pend_fin = attn_qt(0, qt, interleave=thunks,
                                   defer_last_fin=True)
            # proj + qkv PSUM done -> release banks for the P3 pool
            psoA_cm.__exit__(None, None, None)
            psp_cm.__exit__(None, None, None)
            psoB_cm = tc.tile_pool(name="psoB", bufs=3, space="PSUM")
            pso = psoB_cm.__enter__()
            # P3: attn b1; outproj woven between units: b0 rows early
            # (long finalized), the previous tile's b1 rows late
            for qt in range(n_qt):
                units = [(0, rb, ot)
                         for rb in range(qt * opb, (qt + 1) * opb)
                         for ot in range(n_ot)]
                units1 = [(1, rb, ot)
                          for rb in range((qt - 1) * opb, qt * opb)
                          for ot in range(n_ot)] if qt > 0 else []
                last = qt == n_qt - 1
                # on the last tile the late stores' casts alternate onto
                # ACT (its exps are done by then) so the DVE queue can
                # reach the tail finalize's folds sooner
                thunks = ([pend_fin, noop] + op_thunks(units)
                          + op_thunks(units1, alt=last))
                pend_fin = attn_qt(1, qt, interleave=thunks,
                                   defer_last_fin=not last,
                                   fused_tail=last)
            # P4: last query tile's outproj b1; alternate cast engines
            # (ACT is idle here) so the drain is not CAST-bound
            for rb in range((n_qt - 1) * opb, n_qt * opb):
                for ot in range(n_ot):
                    cnt[0] += 1
                    outproj_unit(1, rb, ot, ot % 2)
            psoB_cm.__exit__(None, None, None)

    nc.compile()
    return nc


def _prep_in_maps(x, cos, sin, W_qkv, W_proj, n_cores, hpc, d):
    """Host-side shard prep: pure layout work (transpose / slice / sign fold)."""
    Bp, Tp, Cp = x.shape
    jc = hpc * d
    import ml_dtypes
    tdt = ml_dtypes.bfloat16 if TABLE_DT == "bf16" else np.float32
    xTa = np.ascontiguousarray(x.reshape(Bp * Tp, Cp).T).astype(ml_dtypes.bfloat16)
    cosT = np.ascontiguousarray(cos.T).astype(tdt)
    sinT = np.ascontiguousarray(sin.T).copy()
    sinT[: d // 2] *= -1.0
    sinT = sinT.astype(tdt)
    in_maps = []
    for c in range(n_cores):
        j0, j1 = c * jc, (c + 1) * jc
        in_maps.append({
            "xT": xTa,
            "wqkv": np.ascontiguousarray(np.concatenate(
                [W_qkv[:, j0:j1], W_qkv[:, Cp + j0:Cp + j1],
                 W_qkv[:, 2 * Cp + j0:2 * Cp + j1]], axis=1,
            )).astype(ml_dtypes.bfloat16),
            "wp": np.ascontiguousarray(W_proj[j0:j1, :]).astype(ml_dtypes.bfloat16),
            "ones": np.ones((128, 1), dtype=ml_dtypes.bfloat16),
            "cosT": cosT,
            "sinT": sinT,
        })
    return in_maps


def _install_ntff_hook():
    """Enable NTFF profiling under axon when the boot image lacks the
    antenv.axon_hooks shim. Harmless if anything is missing."""
    import sys
    import types
    try:
        from antenv.axon_hooks import get_axon_ntff_profile_hook
        if get_axon_ntff_profile_hook() is not None:
            return
    except ImportError:
        pass
    try:
        sys.path.insert(0, "/root/.axon_site")
        from trn_agent_boot.trn_boot import _ntff_profile_via_ctypes

        hook = _ntff_profile_via_ctypes("/opt/axon/libaxon_pjrt.so")
        if hook is None:
            return
        mod = types.ModuleType("antenv.axon_hooks")
        mod.get_axon_ntff_profile_hook = lambda: hook
        mod.set_axon_ntff_profile_hook = lambda h: None
        import antenv
        antenv.axon_hooks = mod
        sys.modules["antenv.axon_hooks"] = mod
    except Exception:
        pass


def _run(x, cos, sin, W_qkv, W_proj, trace=False):
    from concourse.bass_utils import run_bass_kernel_spmd

    if trace:
        _install_ntff_hook()

    x = np.ascontiguousarray(x, dtype=np.float32)
    cos = np.ascontiguousarray(cos, dtype=np.float32)
    sin = np.ascontiguousarray(sin, dtype=np.float32)
    W_qkv = np.ascontiguousarray(W_qkv, dtype=np.float32)
    W_proj = np.ascontiguousarray(W_proj, dtype=np.float32)

    Bp, Tp, Cp = x.shape
    nc = _build(Bp, Tp, Cp, HPC, D)
    in_maps = _prep_in_maps(x, cos, sin, W_qkv, W_proj, N_CORES, HPC, D)
    res = run_bass_kernel_spmd(nc, in_maps, core_ids=list(range(N_CORES)),
                               trace=trace)
    acc = np.zeros((Bp * Tp, Cp), dtype=np.float32)
    for i in range(N_CORES):
        acc += np.asarray(res.results[i]["out"], dtype=np.float32)
    return acc.reshape(Bp, Tp, Cp), res


def kernel(x, cos, sin, W_qkv, W_proj):
    out, _ = _run(x, cos, sin, W_qkv, W_proj, trace=False)
    return out

